# revision 3
# baseline (speedup 1.0000x reference)
"""Trainium2 Bass kernel for nn_NodeEdgeConv (GNN message passing).

Strategy (destination-sharded, matmul segment-sum):
- Algebraic reduction: segment_sum(h[idx]*(v@W+b), idx)[n]
    = h[n] * (segment_sum(v, idx)[n] @ W + cnt[n]*b),
  so only the [E, 64] edge payloads need a device-side segment sum; all
  matmuls collapse to node-level GEMMs.
- Edges are sharded by DESTINATION node (node >> 10 -> core), so each core
  computes complete segment sums for its own 1024+1024 node shard and no
  collective is needed at all.
- Host-side degree-sorted slotting: per (core, side), nodes are sorted by
  edge count and assigned to (block, partition) slots; edge payloads are
  laid out in bf16 tiles of [128 tokens, 64] where partition p always
  belongs to node slot p of the current block. The device-side segment sum
  is then just `psum += tile` -- a matmul with a constant identity
  stationary operand. Zero per-edge index processing on device.
- Per block of 128 nodes, the finish (Linear+LayerNorm+Linear residual)
  runs on-chip; outputs are written in permuted order and unscrambled on
  the host.
"""

import numpy as np
import ml_dtypes

import concourse.bass as bass
import concourse.bacc as bacc
import concourse.mybir as mybir
import concourse.tile as tile
from concourse.masks import make_identity

F32 = mybir.dt.float32
BF16 = mybir.dt.bfloat16
BF16_NP = ml_dtypes.bfloat16


class Cfg:
    def __init__(self):
        self.N = 8192          # nodes per side
        self.E = 524288        # edges per type
        self.D = 128
        self.M = 64
        self.C = 8             # cores
        self.NSH = self.N // self.C      # 1024 nodes per core per side
        self.NB = self.NSH // 128        # 8 blocks per side
        self.CH = 128          # tiles per DMA chunk (2 MB bf16)


# ---------------- host-side schedule + layout ----------------

def host_prep(inputs, cfg):
    """Shard edges by destination, degree-sort nodes into (block, partition)
    slots, lay out payload tiles. Returns (in_maps, sched, TOT)."""
    C, NSH, NB, M, CH = cfg.C, cfg.NSH, cfg.NB, cfg.M, cfg.CH

    sides = [
        (np.asarray(inputs["e_s2d_dst"]), np.asarray(inputs["v_s2d"], np.float32)),
        (np.asarray(inputs["e_d2s_dst"]), np.asarray(inputs["v_d2s"], np.float32)),
    ]

    # Per (core, side): local node counts, degree-sorted order.
    percore = [[None] * 2 for _ in range(C)]
    for s, (idx_all, v_all) in enumerate(sides):
        core_of = idx_all // NSH
        for c in range(C):
            esel = np.flatnonzero(core_of == c)
            loc = idx_all[esel] - c * NSH
            cnt = np.bincount(loc, minlength=NSH)
            order = np.argsort(-cnt, kind="stable")     # block b gets order[128b:128b+128]
            percore[c][s] = (esel, loc, cnt, order)

    # SPMD envelope: T[s][b] = max over cores of the block's max count
    # (= count of the block's first node, since sorted descending).
    T = np.zeros((2, NB), np.int64)
    for s in range(2):
        for c in range(C):
            cnt, order = percore[c][s][2], percore[c][s][3]
            sc = cnt[order]
            for b in range(NB):
                T[s][b] = max(T[s][b], sc[128 * b])
    T = np.maximum(T, 1)
    sched = tuple(int(x) for x in T.reshape(-1))
    block_off = np.zeros((2, NB), np.int64)    # tile offset of each block
    off = 0
    for s in range(2):
        for b in range(NB):
            block_off[s][b] = off
            off += T[s][b]
    TILES = off
    NCHUNK = -(-TILES // CH)
    TOT = NCHUNK * CH

    semb = np.asarray(inputs["src_embed"], np.float32)
    demb = np.asarray(inputs["dst_embed"], np.float32)
    emb_by_side = [demb, semb]     # side 0 (s2d) -> dst nodes, side 1 -> src

    weights = {k: np.ascontiguousarray(np.asarray(inputs[k], np.float32)) for k in [
        "W_src", "b_src", "W_dst", "b_dst", "W_sm", "b_sm", "W_dm", "b_dm",
        "row_W1", "row_b1", "row_g", "row_beta", "row_W2", "row_b2",
        "col_W1", "col_b1", "col_g", "col_beta", "col_W2", "col_b2"]}
    identb = np.eye(128, dtype=BF16_NP)

    in_maps = []
    perms = []
    for c in range(C):
        vtiles = np.zeros((TOT, 128, M), BF16_NP)
        m = {"identb": identb}
        cntT = np.zeros((128, 2 * NB), np.float32)
        ords = []
        for s, (idx_all, v_all) in enumerate(sides):
            esel, loc, cnt, order = percore[c][s]
            ords.append(order)
            # slot position of each node
            pos = np.empty(NSH, np.int64)
            pos[order] = np.arange(NSH)
            # group edges by node: stable sort by local node id
            eorder = np.argsort(loc, kind="stable")
            starts = np.zeros(NSH + 1, np.int64)
            np.cumsum(cnt, out=starts[1:])
            rank = np.arange(len(eorder)) - starts[loc[eorder]]
            p_of = pos[loc[eorder]]
            tile_of = block_off[s][p_of // 128] + rank
            flat = tile_of * 128 + (p_of % 128)
            vtiles.reshape(-1, M)[flat] = v_all[esel[eorder]].astype(BF16_NP)
            cntT[:, s * NB:(s + 1) * NB] = cnt[order].reshape(NB, 128).T
            # permuted embeddings for this side
            emb = emb_by_side[s][c * NSH:(c + 1) * NSH][order]
            key = "demb" if s == 0 else "semb"
            m[key] = np.ascontiguousarray(emb)
            m[key + "T"] = np.ascontiguousarray(emb.T)
        # chunk interleave: [NCHUNK, CH, 128, M] -> [NCHUNK, 128, CH*M]
        m["vhw"] = np.ascontiguousarray(
            vtiles.reshape(NCHUNK, CH, 128, M).transpose(0, 2, 1, 3)
            .reshape(NCHUNK, 128, CH * M))
        m["cntT"] = cntT
        m.update(weights)
        in_maps.append(m)
        perms.append(ords)
    return in_maps, sched, TOT, perms


# ---------------- device kernel ----------------

def build_kernel(cfg, sched, TOT):
    C, D, M, NSH, NB, CH = cfg.C, cfg.D, cfg.M, cfg.NSH, cfg.NB, cfg.CH
    T = np.asarray(sched, np.int64).reshape(2, NB)
    NCHUNK = TOT // CH
    nc = bacc.Bacc("TRN2", target_bir_lowering=False, debug=False, num_devices=C)

    vhw = nc.dram_tensor("vhw", [NCHUNK, 128, CH * M], BF16, kind="ExternalInput")
    identb_d = nc.dram_tensor("identb", [128, 128], BF16, kind="ExternalInput")
    semb = nc.dram_tensor("semb", [NSH, D], F32, kind="ExternalInput")
    demb = nc.dram_tensor("demb", [NSH, D], F32, kind="ExternalInput")
    sembT = nc.dram_tensor("sembT", [D, NSH], F32, kind="ExternalInput")
    dembT = nc.dram_tensor("dembT", [D, NSH], F32, kind="ExternalInput")
    cnt = nc.dram_tensor("cntT", [128, 2 * NB], F32, kind="ExternalInput")
    wt = {}
    for k, shp in [
        ("W_src", [D, D]), ("b_src", [D]), ("W_dst", [D, D]), ("b_dst", [D]),
        ("W_sm", [M, D]), ("b_sm", [D]), ("W_dm", [M, D]), ("b_dm", [D]),
        ("row_W1", [D, D]), ("row_b1", [D]), ("row_g", [D]), ("row_beta", [D]),
        ("row_W2", [D, D]), ("row_b2", [D]),
        ("col_W1", [D, D]), ("col_b1", [D]), ("col_g", [D]), ("col_beta", [D]),
        ("col_W2", [D, D]), ("col_b2", [D]),
    ]:
        wt[k] = nc.dram_tensor(k, shp, F32, kind="ExternalInput")
    rowo = nc.dram_tensor("rowo", [NSH, D], F32, kind="ExternalOutput")
    colo = nc.dram_tensor("colo", [NSH, D], F32, kind="ExternalOutput")

    with tile.TileContext(nc) as tc:
        with (
            tc.tile_pool(name="const", bufs=1) as const,
            tc.tile_pool(name="io", bufs=3) as io,
            tc.tile_pool(name="fin", bufs=3) as fin,
            tc.tile_pool(name="psA", bufs=2, space="PSUM") as psa,
            tc.tile_pool(name="ps", bufs=3, space="PSUM") as ps,
        ):
            # --- constants ---
            identb = const.tile([128, 128], BF16)
            nc.sync.dma_start(identb[:], identb_d.ap())
            ident = const.tile([128, 128], F32)
            make_identity(nc, ident[:])
            eps = const.tile([128, 1], F32)
            nc.vector.memset(eps[:], 1e-5)

            def load_w(name, shp):
                t = const.tile(shp, F32, tag=f"w_{name}")
                nc.sync.dma_start(t[:], wt[name].ap())
                return t

            def load_rep(name):
                t = const.tile([128, D], F32, tag=f"rep_{name}")
                b = wt[name].ap()
                nc.gpsimd.dma_start(
                    t[:], bass.AP(tensor=b.tensor, offset=b.offset,
                                  ap=[[0, 128]] + list(b.ap)))
                return t

            Wm_side = {"col": load_w("W_sm", [M, D]), "row": load_w("W_dm", [M, D])}
            bm_side = {"col": load_rep("b_sm"), "row": load_rep("b_dm")}
            W_side = {"col": load_w("W_dst", [D, D]), "row": load_w("W_src", [D, D])}
            b_side = {"col": load_rep("b_dst"), "row": load_rep("b_src")}
            W1 = {"col": load_w("col_W1", [D, D]), "row": load_w("row_W1", [D, D])}
            b1 = {"col": load_rep("col_b1"), "row": load_rep("row_b1")}
            g_ = {"col": load_rep("col_g"), "row": load_rep("row_g")}
            be = {"col": load_rep("col_beta"), "row": load_rep("row_beta")}
            W2 = {"col": load_w("col_W2", [D, D]), "row": load_w("row_W2", [D, D])}
            b2 = {"col": load_rep("col_b2"), "row": load_rep("row_b2")}

            cnt_t = const.tile([128, 2 * NB], F32)
            nc.sync.dma_start(cnt_t[:], cnt.ap())

            # whole-side embeddings resident in SBUF
            embT_sb = {}
            emb_sb = {}
            for key, dT, dE in (("col", dembT, demb), ("row", sembT, semb)):
                tT = const.tile([128, NSH], F32, tag=f"embT_{key}")
                nc.sync.dma_start(tT[:], dT.ap())
                embT_sb[key] = tT
                tE = const.tile([128, NB, D], F32, tag=f"emb_{key}")
                nc.sync.dma_start(
                    tE[:], dE.ap().rearrange("(b p) d -> p b d", p=128))
                emb_sb[key] = tE

            # --- main loop: stream chunks, identity-matmul segment sums,
            #     per-block finish ---
            cur_chunk = [None, -1]

            def chunk_for(tidx):
                cidx = tidx // CH
                if cur_chunk[1] != cidx:
                    t = io.tile([128, CH, M], BF16, tag="vchunk")
                    nc.sync.dma_start(
                        t[:], vhw.ap()[cidx].rearrange("p (t m) -> p t m", m=M))
                    cur_chunk[0], cur_chunk[1] = t, cidx
                return cur_chunk[0]

            tidx = 0
            for s, side in ((0, "col"), (1, "row")):
                out_d = colo if side == "col" else rowo
                for b in range(NB):
                    Tb = int(T[s][b])
                    A_ps = psa.tile([128, M], F32, tag="A")
                    for t in range(Tb):
                        ck = chunk_for(tidx)
                        nc.tensor.matmul(
                            A_ps[:], lhsT=identb[:], rhs=ck[:, tidx % CH, :],
                            start=(t == 0), stop=(t == Tb - 1),
                            skip_group_check=True)
                        tidx += 1

                    # ---- finish for this block of 128 nodes ----
                    n0 = b * 128
                    ET = embT_sb[side][:, n0:n0 + 128]
                    E_t = emb_sb[side][:, b, :]
                    A_t = fin.tile([128, M], F32, tag="A")
                    nc.vector.tensor_copy(A_t[:], A_ps[:])

                    # h = emb @ W + b
                    h_ps = ps.tile([128, D], F32, tag="p1")
                    nc.tensor.matmul(h_ps[:], lhsT=ET, rhs=W_side[side][:])
                    h = fin.tile([128, D], F32, tag="h")
                    nc.vector.tensor_add(h[:], h_ps[:], b_side[side][:])

                    # S = A @ Wm + cnt*bm
                    at_ps = ps.tile([M, 128], F32, tag="p2")
                    nc.tensor.transpose(at_ps[:], A_t[:], ident[:])
                    AT = fin.tile([M, 128], F32, tag="AT")
                    nc.vector.tensor_copy(AT[:], at_ps[:])
                    s_ps = ps.tile([128, D], F32, tag="p1")
                    nc.tensor.matmul(s_ps[:], lhsT=AT[:], rhs=Wm_side[side][:])
                    cb = fin.tile([128, D], F32, tag="cb")
                    nc.vector.tensor_scalar_mul(
                        cb[:], in0=bm_side[side][:],
                        scalar1=cnt_t[:, s * NB + b:s * NB + b + 1])
                    S = fin.tile([128, D], F32, tag="S")
                    nc.vector.tensor_add(S[:], s_ps[:], cb[:])

                    # u = h * S
                    u = fin.tile([128, D], F32, tag="u")
                    nc.vector.tensor_mul(u[:], h[:], S[:])

                    # t1 = u @ W1 + b1
                    ut_ps = ps.tile([128, 128], F32, tag="p2")
                    nc.tensor.transpose(ut_ps[:], u[:], ident[:])
                    uT = fin.tile([128, 128], F32, tag="uT")
                    nc.vector.tensor_copy(uT[:], ut_ps[:])
                    t1_ps = ps.tile([128, D], F32, tag="p1")
                    nc.tensor.matmul(t1_ps[:], lhsT=uT[:], rhs=W1[side][:])
                    t1 = fin.tile([128, D], F32, tag="t1")
                    nc.vector.tensor_add(t1[:], t1_ps[:], b1[side][:])

                    # LN(t1) * g + beta
                    stats = fin.tile([128, nc.vector.BN_STATS_DIM], F32, tag="st")
                    nc.vector.bn_stats(stats[:], t1[:])
                    mv = fin.tile([128, nc.vector.BN_AGGR_DIM], F32, tag="mv")
                    nc.vector.bn_aggr(mv[:], stats[:])
                    rstd = fin.tile([128, 1], F32, tag="rs")
                    nc.scalar.activation(
                        rstd[:], mv[:, 1:2],
                        func=mybir.ActivationFunctionType.Sqrt,
                        bias=eps[:], scale=1.0, alpha=0.0)
                    nc.vector.reciprocal(rstd[:], rstd[:])
                    nc.vector.tensor_scalar(
                        t1[:], in0=t1[:], scalar1=mv[:, 0:1], scalar2=rstd[:],
                        op0=mybir.AluOpType.subtract, op1=mybir.AluOpType.mult)
                    nc.vector.tensor_mul(t1[:], t1[:], g_[side][:])
                    nc.vector.tensor_add(t1[:], t1[:], be[side][:])

                    # t2 = ln @ W2 + b2 ; out = emb + t2
                    lt_ps = ps.tile([128, 128], F32, tag="p2")
                    nc.tensor.transpose(lt_ps[:], t1[:], ident[:])
                    lT = fin.tile([128, 128], F32, tag="lT")
                    nc.vector.tensor_copy(lT[:], lt_ps[:])
                    t2_ps = ps.tile([128, D], F32, tag="p1")
                    nc.tensor.matmul(t2_ps[:], lhsT=lT[:], rhs=W2[side][:])
                    ot = fin.tile([128, D], F32, tag="ot")
                    nc.vector.tensor_add(ot[:], t2_ps[:], b2[side][:])
                    nc.vector.tensor_add(ot[:], ot[:], E_t)
                    nc.sync.dma_start(out_d.ap()[n0:n0 + 128, :], ot[:])

    nc.compile()
    return nc


def assemble(results, perms, cfg):
    NSH = cfg.NSH
    row = np.empty((cfg.N, cfg.D), np.float32)
    col = np.empty((cfg.N, cfg.D), np.float32)
    for c, r in enumerate(results):
        ord_s2d, ord_d2s = perms[c]
        col[c * NSH + ord_s2d] = r["colo"]
        row[c * NSH + ord_d2s] = r["rowo"]
    return row, col


# ---------------- graded entry point ----------------

_CACHE = {}


def kernel(**inputs):
    cfg = Cfg()
    in_maps, sched, TOT, perms = host_prep(inputs, cfg)
    key = (sched, TOT)
    if key not in _CACHE:
        _CACHE[key] = build_kernel(cfg, sched, TOT)
    nc = _CACHE[key]
    from concourse.bass_utils import run_bass_kernel_spmd
    res = run_bass_kernel_spmd(nc, in_maps, core_ids=list(range(cfg.C)))
    return assemble(res.results, perms, cfg)


# revision 6
# speedup vs baseline: 21.6432x; 21.6432x over previous
"""Trainium2 Bass kernel for nn_NodeEdgeConv (GNN message passing).

Strategy (destination-sharded, matmul segment-sum):
- Algebraic reduction: segment_sum(h[idx]*(v@W+b), idx)[n]
    = h[n] * (segment_sum(v, idx)[n] @ W + cnt[n]*b),
  so only the [E, 64] edge payloads need a device-side segment sum; all
  matmuls collapse to node-level GEMMs.
- Edges are sharded by DESTINATION node (node >> 10 -> core), so each core
  computes complete segment sums for its own 1024+1024 node shard and no
  collective is needed at all.
- Host-side degree-sorted slotting: per (core, side), nodes are sorted by
  edge count and assigned to (block, partition) slots; edge payloads are
  laid out in bf16 tiles of [128 tokens, 64] where partition p always
  belongs to node slot p of the current block. The device-side segment sum
  is then just `psum += tile` -- a matmul with a constant identity
  stationary operand. Zero per-edge index processing on device.
- Per block of 128 nodes, the finish (Linear+LayerNorm+Linear residual)
  runs on-chip; outputs are written in permuted order and unscrambled on
  the host.
"""

import numpy as np
import ml_dtypes

import concourse.bass as bass
import concourse.bacc as bacc
import concourse.mybir as mybir
import concourse.tile as tile
from concourse.masks import make_identity

F32 = mybir.dt.float32
BF16 = mybir.dt.bfloat16
BF16_NP = ml_dtypes.bfloat16


class Cfg:
    def __init__(self):
        self.N = 8192          # nodes per side
        self.E = 524288        # edges per type
        self.D = 128
        self.M = 64
        self.C = 8             # cores
        self.NSH = self.N // self.C      # 1024 nodes per core per side
        self.NB = self.NSH // 128        # 8 blocks per side
        self.CH = 128          # tiles per DMA chunk (2 MB bf16)


# ---------------- host-side schedule + layout ----------------

def host_prep(inputs, cfg):
    """Shard edges by destination, degree-sort nodes into (block, partition)
    slots, lay out payload tiles. Returns (in_maps, sched, TOT)."""
    C, NSH, NB, M, CH = cfg.C, cfg.NSH, cfg.NB, cfg.M, cfg.CH

    sides = [
        (np.asarray(inputs["e_s2d_dst"]), np.asarray(inputs["v_s2d"], np.float32)),
        (np.asarray(inputs["e_d2s_dst"]), np.asarray(inputs["v_d2s"], np.float32)),
    ]

    # Per (core, side): local node counts, degree-sorted order.
    percore = [[None] * 2 for _ in range(C)]
    for s, (idx_all, v_all) in enumerate(sides):
        core_of = idx_all // NSH
        for c in range(C):
            esel = np.flatnonzero(core_of == c)
            loc = idx_all[esel] - c * NSH
            cnt = np.bincount(loc, minlength=NSH)
            order = np.argsort(-cnt, kind="stable")     # block b gets order[128b:128b+128]
            percore[c][s] = (esel, loc, cnt, order)

    # SPMD envelope: T[s][b] = max over cores of the block's max count
    # (= count of the block's first node, since sorted descending).
    T = np.zeros((2, NB), np.int64)
    for s in range(2):
        for c in range(C):
            cnt, order = percore[c][s][2], percore[c][s][3]
            sc = cnt[order]
            for b in range(NB):
                T[s][b] = max(T[s][b], sc[128 * b])
    T = np.maximum(T, 1)
    sched = tuple(int(x) for x in T.reshape(-1))
    block_off = np.zeros((2, NB), np.int64)    # tile offset of each block
    off = 0
    for s in range(2):
        for b in range(NB):
            block_off[s][b] = off
            off += T[s][b]
    TILES = off
    NCHUNK = -(-TILES // CH)
    TOT = NCHUNK * CH

    semb = np.asarray(inputs["src_embed"], np.float32)
    demb = np.asarray(inputs["dst_embed"], np.float32)
    emb_by_side = [demb, semb]     # side 0 (s2d) -> dst nodes, side 1 -> src

    weights = {k: np.ascontiguousarray(np.asarray(inputs[k], np.float32)) for k in [
        "W_src", "b_src", "W_dst", "b_dst", "W_sm", "b_sm", "W_dm", "b_dm",
        "row_W1", "row_b1", "row_g", "row_beta", "row_W2", "row_b2",
        "col_W1", "col_b1", "col_g", "col_beta", "col_W2", "col_b2"]}
    identb = np.eye(128, dtype=BF16_NP)

    in_maps = []
    perms = []
    for c in range(C):
        vtiles = np.zeros((TOT, 128, M), BF16_NP)
        m = {"identb": identb}
        cntT = np.zeros((128, 2 * NB), np.float32)
        ords = []
        for s, (idx_all, v_all) in enumerate(sides):
            esel, loc, cnt, order = percore[c][s]
            ords.append(order)
            # slot position of each node
            pos = np.empty(NSH, np.int64)
            pos[order] = np.arange(NSH)
            # group edges by node: stable sort by local node id
            eorder = np.argsort(loc, kind="stable")
            starts = np.zeros(NSH + 1, np.int64)
            np.cumsum(cnt, out=starts[1:])
            rank = np.arange(len(eorder)) - starts[loc[eorder]]
            p_of = pos[loc[eorder]]
            tile_of = block_off[s][p_of // 128] + rank
            flat = tile_of * 128 + (p_of % 128)
            vtiles.reshape(-1, M)[flat] = v_all[esel[eorder]].astype(BF16_NP)
            cntT[:, s * NB:(s + 1) * NB] = cnt[order].reshape(NB, 128).T
            # permuted embeddings for this side
            emb = emb_by_side[s][c * NSH:(c + 1) * NSH][order]
            key = "demb" if s == 0 else "semb"
            m[key] = np.ascontiguousarray(emb)
            m[key + "T"] = np.ascontiguousarray(emb.T)
        # chunk interleave: [NCHUNK, CH, 128, M] -> [NCHUNK, 128, CH*M]
        m["vhw"] = np.ascontiguousarray(
            vtiles.reshape(NCHUNK, CH, 128, M).transpose(0, 2, 1, 3)
            .reshape(NCHUNK, 128, CH * M))
        m["cntT"] = cntT
        m.update(weights)
        in_maps.append(m)
        perms.append(ords)
    return in_maps, sched, TOT, perms


# ---------------- device kernel ----------------

def build_kernel(cfg, sched, TOT, reps=1):
    import contextlib
    C, D, M, NSH, NB, CH = cfg.C, cfg.D, cfg.M, cfg.NSH, cfg.NB, cfg.CH
    T = np.asarray(sched, np.int64).reshape(2, NB)
    NCHUNK = TOT // CH
    nc = bacc.Bacc("TRN2", target_bir_lowering=False, debug=False, num_devices=C)

    vhw = nc.dram_tensor("vhw", [NCHUNK, 128, CH * M], BF16, kind="ExternalInput")
    identb_d = nc.dram_tensor("identb", [128, 128], BF16, kind="ExternalInput")
    semb = nc.dram_tensor("semb", [NSH, D], F32, kind="ExternalInput")
    demb = nc.dram_tensor("demb", [NSH, D], F32, kind="ExternalInput")
    sembT = nc.dram_tensor("sembT", [D, NSH], F32, kind="ExternalInput")
    dembT = nc.dram_tensor("dembT", [D, NSH], F32, kind="ExternalInput")
    cnt = nc.dram_tensor("cntT", [128, 2 * NB], F32, kind="ExternalInput")
    wt = {}
    for k, shp in [
        ("W_src", [D, D]), ("b_src", [D]), ("W_dst", [D, D]), ("b_dst", [D]),
        ("W_sm", [M, D]), ("b_sm", [D]), ("W_dm", [M, D]), ("b_dm", [D]),
        ("row_W1", [D, D]), ("row_b1", [D]), ("row_g", [D]), ("row_beta", [D]),
        ("row_W2", [D, D]), ("row_b2", [D]),
        ("col_W1", [D, D]), ("col_b1", [D]), ("col_g", [D]), ("col_beta", [D]),
        ("col_W2", [D, D]), ("col_b2", [D]),
    ]:
        wt[k] = nc.dram_tensor(k, shp, F32, kind="ExternalInput")
    rowo = nc.dram_tensor("rowo", [NSH, D], F32, kind="ExternalOutput")
    colo = nc.dram_tensor("colo", [NSH, D], F32, kind="ExternalOutput")

    with tile.TileContext(nc) as tc:
        with (
            tc.tile_pool(name="const", bufs=1) as const,
            tc.tile_pool(name="io", bufs=3) as io,
            tc.tile_pool(name="fin", bufs=3) as fin,
            tc.tile_pool(name="psA", bufs=2, space="PSUM") as psa,
            tc.tile_pool(name="ps", bufs=3, space="PSUM") as ps,
        ):
            # --- constants ---
            identb = const.tile([128, 128], BF16)
            nc.sync.dma_start(identb[:], identb_d.ap())
            ident = const.tile([128, 128], F32)
            make_identity(nc, ident[:])
            eps = const.tile([128, 1], F32)
            nc.vector.memset(eps[:], 1e-5)

            def load_w(name, shp):
                t = const.tile(shp, F32, tag=f"w_{name}")
                nc.sync.dma_start(t[:], wt[name].ap())
                return t

            def load_rep(name):
                t = const.tile([128, D], F32, tag=f"rep_{name}")
                b = wt[name].ap()
                nc.gpsimd.dma_start(
                    t[:], bass.AP(tensor=b.tensor, offset=b.offset,
                                  ap=[[0, 128]] + list(b.ap)))
                return t

            Wm_side = {"col": load_w("W_sm", [M, D]), "row": load_w("W_dm", [M, D])}
            bm_side = {"col": load_rep("b_sm"), "row": load_rep("b_dm")}
            W_side = {"col": load_w("W_dst", [D, D]), "row": load_w("W_src", [D, D])}
            b_side = {"col": load_rep("b_dst"), "row": load_rep("b_src")}
            W1 = {"col": load_w("col_W1", [D, D]), "row": load_w("row_W1", [D, D])}
            b1 = {"col": load_rep("col_b1"), "row": load_rep("row_b1")}
            g_ = {"col": load_rep("col_g"), "row": load_rep("row_g")}
            be = {"col": load_rep("col_beta"), "row": load_rep("row_beta")}
            W2 = {"col": load_w("col_W2", [D, D]), "row": load_w("row_W2", [D, D])}
            b2 = {"col": load_rep("col_b2"), "row": load_rep("row_b2")}

            cnt_t = const.tile([128, 2 * NB], F32)
            nc.sync.dma_start(cnt_t[:], cnt.ap())

            # whole-side embeddings resident in SBUF
            embT_sb = {}
            emb_sb = {}
            for key, dT, dE in (("col", dembT, demb), ("row", sembT, semb)):
                tT = const.tile([128, NSH], F32, tag=f"embT_{key}")
                nc.sync.dma_start(tT[:], dT.ap())
                embT_sb[key] = tT
                tE = const.tile([128, NB, D], F32, tag=f"emb_{key}")
                nc.sync.dma_start(
                    tE[:], dE.ap().rearrange("(b p) d -> p b d", p=128))
                emb_sb[key] = tE

            # --- main loop: stream chunks, identity-matmul segment sums,
            #     per-block finish ---
            rep_ctx = tc.For_i(0, reps) if reps > 1 else contextlib.nullcontext()
            with rep_ctx:
                run_body(nc, tc, cfg, T, io, fin, psa, ps, vhw, identb, ident,
                         eps, Wm_side, bm_side, W_side, b_side, W1, b1, g_, be,
                         W2, b2, cnt_t, embT_sb, emb_sb, colo, rowo)

    nc.compile()
    return nc


def run_body(nc, tc, cfg, T, io, fin, psa, ps, vhw, identb, ident, eps,
             Wm_side, bm_side, W_side, b_side, W1, b1, g_, be, W2, b2,
             cnt_t, embT_sb, emb_sb, colo, rowo):
    D, M, NB, CH = cfg.D, cfg.M, cfg.NB, cfg.CH
    F32 = mybir.dt.float32
    BF16 = mybir.dt.bfloat16
    if True:
            cur_chunk = [None, -1]

            def chunk_for(tidx):
                cidx = tidx // CH
                if cur_chunk[1] != cidx:
                    t = io.tile([128, CH, M], BF16, tag="vchunk")
                    nc.sync.dma_start(
                        t[:], vhw.ap()[cidx].rearrange("p (t m) -> p t m", m=M))
                    cur_chunk[0], cur_chunk[1] = t, cidx
                return cur_chunk[0]

            tidx = 0
            for s, side in ((0, "col"), (1, "row")):
                out_d = colo if side == "col" else rowo
                for b in range(NB):
                    Tb = int(T[s][b])
                    A_ps = psa.tile([128, M], F32, tag="A")
                    for t in range(Tb):
                        ck = chunk_for(tidx)
                        nc.tensor.matmul(
                            A_ps[:], lhsT=identb[:], rhs=ck[:, tidx % CH, :],
                            start=(t == 0), stop=(t == Tb - 1),
                            skip_group_check=True)
                        tidx += 1

                    # ---- finish for this block of 128 nodes ----
                    n0 = b * 128
                    ET = embT_sb[side][:, n0:n0 + 128]
                    E_t = emb_sb[side][:, b, :]
                    A_t = fin.tile([128, M], F32, tag="A")
                    nc.vector.tensor_copy(A_t[:], A_ps[:])

                    # h = emb @ W + b
                    h_ps = ps.tile([128, D], F32, tag="p1")
                    nc.tensor.matmul(h_ps[:], lhsT=ET, rhs=W_side[side][:])
                    h = fin.tile([128, D], F32, tag="h")
                    nc.vector.tensor_add(h[:], h_ps[:], b_side[side][:])

                    # S = A @ Wm + cnt*bm
                    at_ps = ps.tile([M, 128], F32, tag="p2")
                    nc.tensor.transpose(at_ps[:], A_t[:], ident[:])
                    AT = fin.tile([M, 128], F32, tag="AT")
                    nc.vector.tensor_copy(AT[:], at_ps[:])
                    s_ps = ps.tile([128, D], F32, tag="p1")
                    nc.tensor.matmul(s_ps[:], lhsT=AT[:], rhs=Wm_side[side][:])
                    cb = fin.tile([128, D], F32, tag="cb")
                    nc.vector.tensor_scalar_mul(
                        cb[:], in0=bm_side[side][:],
                        scalar1=cnt_t[:, s * NB + b:s * NB + b + 1])
                    S = fin.tile([128, D], F32, tag="S")
                    nc.vector.tensor_add(S[:], s_ps[:], cb[:])

                    # u = h * S
                    u = fin.tile([128, D], F32, tag="u")
                    nc.vector.tensor_mul(u[:], h[:], S[:])

                    # t1 = u @ W1 + b1
                    ut_ps = ps.tile([128, 128], F32, tag="p2")
                    nc.tensor.transpose(ut_ps[:], u[:], ident[:])
                    uT = fin.tile([128, 128], F32, tag="uT")
                    nc.vector.tensor_copy(uT[:], ut_ps[:])
                    t1_ps = ps.tile([128, D], F32, tag="p1")
                    nc.tensor.matmul(t1_ps[:], lhsT=uT[:], rhs=W1[side][:])
                    t1 = fin.tile([128, D], F32, tag="t1")
                    nc.vector.tensor_add(t1[:], t1_ps[:], b1[side][:])

                    # LN(t1) * g + beta
                    stats = fin.tile([128, nc.vector.BN_STATS_DIM], F32, tag="st")
                    nc.vector.bn_stats(stats[:], t1[:])
                    mv = fin.tile([128, nc.vector.BN_AGGR_DIM], F32, tag="mv")
                    nc.vector.bn_aggr(mv[:], stats[:])
                    rstd = fin.tile([128, 1], F32, tag="rs")
                    nc.scalar.activation(
                        rstd[:], mv[:, 1:2],
                        func=mybir.ActivationFunctionType.Sqrt,
                        bias=eps[:], scale=1.0, alpha=0.0)
                    nc.vector.reciprocal(rstd[:], rstd[:])
                    nc.vector.tensor_scalar(
                        t1[:], in0=t1[:], scalar1=mv[:, 0:1], scalar2=rstd[:],
                        op0=mybir.AluOpType.subtract, op1=mybir.AluOpType.mult)
                    nc.vector.tensor_mul(t1[:], t1[:], g_[side][:])
                    nc.vector.tensor_add(t1[:], t1[:], be[side][:])

                    # t2 = ln @ W2 + b2 ; out = emb + t2
                    lt_ps = ps.tile([128, 128], F32, tag="p2")
                    nc.tensor.transpose(lt_ps[:], t1[:], ident[:])
                    lT = fin.tile([128, 128], F32, tag="lT")
                    nc.vector.tensor_copy(lT[:], lt_ps[:])
                    t2_ps = ps.tile([128, D], F32, tag="p1")
                    nc.tensor.matmul(t2_ps[:], lhsT=lT[:], rhs=W2[side][:])
                    ot = fin.tile([128, D], F32, tag="ot")
                    nc.vector.tensor_add(ot[:], t2_ps[:], b2[side][:])
                    nc.vector.tensor_add(ot[:], ot[:], E_t)
                    nc.sync.dma_start(out_d.ap()[n0:n0 + 128, :], ot[:])


def assemble(results, perms, cfg):
    NSH = cfg.NSH
    row = np.empty((cfg.N, cfg.D), np.float32)
    col = np.empty((cfg.N, cfg.D), np.float32)
    for c, r in enumerate(results):
        ord_s2d, ord_d2s = perms[c]
        col[c * NSH + ord_s2d] = r["colo"]
        row[c * NSH + ord_d2s] = r["rowo"]
    return row, col


# ---------------- graded entry point ----------------

_CACHE = {}


def kernel(**inputs):
    cfg = Cfg()
    in_maps, sched, TOT, perms = host_prep(inputs, cfg)
    key = (sched, TOT)
    if key not in _CACHE:
        _CACHE[key] = build_kernel(cfg, sched, TOT)
    nc = _CACHE[key]
    from concourse.bass_utils import run_bass_kernel_spmd
    res = run_bass_kernel_spmd(nc, in_maps, core_ids=list(range(cfg.C)))
    return assemble(res.results, perms, cfg)


# revision 12
# speedup vs baseline: 26.2063x; 1.2108x over previous
"""Trainium2 Bass kernel for nn_NodeEdgeConv (GNN message passing).

Strategy (destination-sharded, matmul segment-sum):
- Algebraic reduction: segment_sum(h[idx]*(v@W+b), idx)[n]
    = h[n] * (segment_sum(v, idx)[n] @ W + cnt[n]*b),
  so only the [E, 64] edge payloads need a device-side segment sum; all
  matmuls collapse to node-level GEMMs.
- Edges are sharded by DESTINATION node (node >> 10 -> core), so each core
  computes complete segment sums for its own 1024+1024 node shard and no
  collective is needed at all.
- Host-side degree-sorted slotting: per (core, side), nodes are sorted by
  edge count and assigned to (block, partition) slots; edge payloads are
  laid out in bf16 tiles of [128 tokens, 64] where partition p always
  belongs to node slot p of the current block. The device-side segment sum
  is then just `psum += tile` -- a matmul with a constant identity
  stationary operand. Zero per-edge index processing on device.
- Per block of 128 nodes, the finish (Linear+LayerNorm+Linear residual)
  runs on-chip; outputs are written in permuted order and unscrambled on
  the host.
"""

import numpy as np
import ml_dtypes

import concourse.bass as bass
import concourse.bacc as bacc
import concourse.mybir as mybir
import concourse.tile as tile
from concourse.masks import make_identity

F32 = mybir.dt.float32
BF16 = mybir.dt.bfloat16
BF16_NP = ml_dtypes.bfloat16
F8 = mybir.dt.float8e4
F8_NP = ml_dtypes.float8_e4m3


class Cfg:
    def __init__(self):
        self.N = 8192          # nodes per side
        self.E = 524288        # edges per type
        self.D = 128
        self.M = 64
        self.C = 8             # cores
        self.NSH = self.N // self.C      # 1024 nodes per core per side
        self.NB = self.NSH // 128        # 8 blocks per side
        self.CH = 192          # tiles per DMA chunk (1.5 MB fp8)


# ---------------- host-side schedule + layout ----------------

def host_prep(inputs, cfg):
    """Shard edges by destination, degree-sort nodes into (block, partition)
    slots, lay out payload tiles. Returns (in_maps, sched, TOT)."""
    C, NSH, NB, M, CH = cfg.C, cfg.NSH, cfg.NB, cfg.M, cfg.CH

    sides = [
        (np.asarray(inputs["e_s2d_dst"]), np.asarray(inputs["v_s2d"], np.float32)),
        (np.asarray(inputs["e_d2s_dst"]), np.asarray(inputs["v_d2s"], np.float32)),
    ]

    # Per (core, side): local node counts, degree-sorted order.
    percore = [[None] * 2 for _ in range(C)]
    for s, (idx_all, v_all) in enumerate(sides):
        core_of = idx_all // NSH
        for c in range(C):
            esel = np.flatnonzero(core_of == c)
            loc = idx_all[esel] - c * NSH
            cnt = np.bincount(loc, minlength=NSH)
            order = np.argsort(-cnt, kind="stable")     # block b gets order[128b:128b+128]
            percore[c][s] = (esel, loc, cnt, order)

    # SPMD envelope: T[s][b] = max over cores of the block's max count
    # (= count of the block's first node, since sorted descending).
    T = np.zeros((2, NB), np.int64)
    for s in range(2):
        for c in range(C):
            cnt, order = percore[c][s][2], percore[c][s][3]
            sc = cnt[order]
            for b in range(NB):
                T[s][b] = max(T[s][b], sc[128 * b])
    T = np.maximum(T, 1)
    sched = tuple(int(x) for x in T.reshape(-1))
    block_off = np.zeros((2, NB), np.int64)    # tile offset of each block
    off = 0
    for s in range(2):
        for b in range(NB):
            block_off[s][b] = off
            off += T[s][b]
    TILES = off
    NCHUNK = -(-TILES // CH)
    TOT = NCHUNK * CH

    semb = np.asarray(inputs["src_embed"], np.float32)
    demb = np.asarray(inputs["dst_embed"], np.float32)
    emb_by_side = [demb, semb]     # side 0 (s2d) -> dst nodes, side 1 -> src

    weights = {k: np.ascontiguousarray(np.asarray(inputs[k], np.float32)) for k in [
        "W_src", "b_src", "W_dst", "b_dst", "W_sm", "b_sm", "W_dm", "b_dm",
        "row_W1", "row_b1", "row_g", "row_beta", "row_W2", "row_b2",
        "col_W1", "col_b1", "col_g", "col_beta", "col_W2", "col_b2"]}
    identb = np.eye(128, dtype=F8_NP)

    in_maps = []
    perms = []
    for c in range(C):
        vtiles = np.zeros((TOT, 128, M), F8_NP)
        m = {"identb": identb}
        cntT = np.zeros((128, 2 * NB), np.float32)
        ords = []
        for s, (idx_all, v_all) in enumerate(sides):
            esel, loc, cnt, order = percore[c][s]
            ords.append(order)
            # slot position of each node
            pos = np.empty(NSH, np.int64)
            pos[order] = np.arange(NSH)
            # group edges by node: stable sort by local node id
            eorder = np.argsort(loc, kind="stable")
            starts = np.zeros(NSH + 1, np.int64)
            np.cumsum(cnt, out=starts[1:])
            rank = np.arange(len(eorder)) - starts[loc[eorder]]
            p_of = pos[loc[eorder]]
            tile_of = block_off[s][p_of // 128] + rank
            flat = tile_of * 128 + (p_of % 128)
            vtiles.reshape(-1, M)[flat] = v_all[esel[eorder]].astype(F8_NP)
            cntT[:, s * NB:(s + 1) * NB] = cnt[order].reshape(NB, 128).T
            # permuted embeddings for this side
            emb = emb_by_side[s][c * NSH:(c + 1) * NSH][order]
            key = "demb" if s == 0 else "semb"
            m[key] = np.ascontiguousarray(emb)
            m[key + "T"] = np.ascontiguousarray(emb.T)
        # chunk interleave: [NCHUNK, CH, 128, M] -> [NCHUNK, 128, CH*M]
        m["vhw"] = np.ascontiguousarray(
            vtiles.reshape(NCHUNK, CH, 128, M).transpose(0, 2, 1, 3)
            .reshape(NCHUNK, 128, CH * M))
        m["cntT"] = cntT
        m.update(weights)
        in_maps.append(m)
        perms.append(ords)
    return in_maps, sched, TOT, perms


# ---------------- device kernel ----------------

def build_kernel(cfg, sched, TOT, reps=1, mode="full"):
    import contextlib
    C, D, M, NSH, NB, CH = cfg.C, cfg.D, cfg.M, cfg.NSH, cfg.NB, cfg.CH
    T = np.asarray(sched, np.int64).reshape(2, NB)
    NCHUNK = TOT // CH
    nc = bacc.Bacc("TRN2", target_bir_lowering=False, debug=False, num_devices=C)

    vhw = nc.dram_tensor("vhw", [NCHUNK, 128, CH * M], F8, kind="ExternalInput")
    identb_d = nc.dram_tensor("identb", [128, 128], F8, kind="ExternalInput")
    semb = nc.dram_tensor("semb", [NSH, D], F32, kind="ExternalInput")
    demb = nc.dram_tensor("demb", [NSH, D], F32, kind="ExternalInput")
    sembT = nc.dram_tensor("sembT", [D, NSH], F32, kind="ExternalInput")
    dembT = nc.dram_tensor("dembT", [D, NSH], F32, kind="ExternalInput")
    cnt = nc.dram_tensor("cntT", [128, 2 * NB], F32, kind="ExternalInput")
    wt = {}
    for k, shp in [
        ("W_src", [D, D]), ("b_src", [D]), ("W_dst", [D, D]), ("b_dst", [D]),
        ("W_sm", [M, D]), ("b_sm", [D]), ("W_dm", [M, D]), ("b_dm", [D]),
        ("row_W1", [D, D]), ("row_b1", [D]), ("row_g", [D]), ("row_beta", [D]),
        ("row_W2", [D, D]), ("row_b2", [D]),
        ("col_W1", [D, D]), ("col_b1", [D]), ("col_g", [D]), ("col_beta", [D]),
        ("col_W2", [D, D]), ("col_b2", [D]),
    ]:
        wt[k] = nc.dram_tensor(k, shp, F32, kind="ExternalInput")
    rowo = nc.dram_tensor("rowo", [NSH, D], F32, kind="ExternalOutput")
    colo = nc.dram_tensor("colo", [NSH, D], F32, kind="ExternalOutput")

    with tile.TileContext(nc) as tc:
        with (
            tc.tile_pool(name="const", bufs=1) as const,
            tc.tile_pool(name="io", bufs=3) as io,
            tc.tile_pool(name="fin", bufs=3) as fin,
            tc.tile_pool(name="psA", bufs=2, space="PSUM") as psa,
            tc.tile_pool(name="ps", bufs=3, space="PSUM") as ps,
        ):
            # --- constants ---
            identb = const.tile([128, 128], F8)
            nc.sync.dma_start(identb[:], identb_d.ap())
            ident = const.tile([128, 128], F32)
            make_identity(nc, ident[:])
            eps = const.tile([128, 1], F32)
            nc.vector.memset(eps[:], 1e-5)

            def load_w(name, shp):
                t = const.tile(shp, F32, tag=f"w_{name}")
                nc.sync.dma_start(t[:], wt[name].ap())
                return t

            def load_rep(name):
                t = const.tile([128, D], F32, tag=f"rep_{name}")
                b = wt[name].ap()
                nc.gpsimd.dma_start(
                    t[:], bass.AP(tensor=b.tensor, offset=b.offset,
                                  ap=[[0, 128]] + list(b.ap)))
                return t

            Wm_side = {"col": load_w("W_sm", [M, D]), "row": load_w("W_dm", [M, D])}
            bm_side = {"col": load_rep("b_sm"), "row": load_rep("b_dm")}
            W_side = {"col": load_w("W_dst", [D, D]), "row": load_w("W_src", [D, D])}
            b_side = {"col": load_rep("b_dst"), "row": load_rep("b_src")}
            W1 = {"col": load_w("col_W1", [D, D]), "row": load_w("row_W1", [D, D])}
            b1 = {"col": load_rep("col_b1"), "row": load_rep("row_b1")}
            g_ = {"col": load_rep("col_g"), "row": load_rep("row_g")}
            be = {"col": load_rep("col_beta"), "row": load_rep("row_beta")}
            W2 = {"col": load_w("col_W2", [D, D]), "row": load_w("row_W2", [D, D])}
            b2 = {"col": load_rep("col_b2"), "row": load_rep("row_b2")}

            cnt_t = const.tile([128, 2 * NB], F32)
            nc.sync.dma_start(cnt_t[:], cnt.ap())

            # whole-side embeddings resident in SBUF
            embT_sb = {}
            emb_sb = {}
            for key, dT, dE in (("col", dembT, demb), ("row", sembT, semb)):
                tT = const.tile([128, NSH], F32, tag=f"embT_{key}")
                nc.sync.dma_start(tT[:], dT.ap())
                embT_sb[key] = tT
                tE = const.tile([128, NB, D], F32, tag=f"emb_{key}")
                nc.sync.dma_start(
                    tE[:], dE.ap().rearrange("(b p) d -> p b d", p=128))
                emb_sb[key] = tE

            # --- main loop: stream chunks, identity-matmul segment sums,
            #     per-block finish ---
            rep_ctx = tc.For_i(0, reps) if reps > 1 else contextlib.nullcontext()
            with rep_ctx:
                run_body(nc, tc, cfg, T, io, fin, psa, ps, vhw, identb, ident,
                         eps, Wm_side, bm_side, W_side, b_side, W1, b1, g_, be,
                         W2, b2, cnt_t, embT_sb, emb_sb, colo, rowo, mode)

    nc.compile()
    return nc


def run_body(nc, tc, cfg, T, io, fin, psa, ps, vhw, identb, ident, eps,
             Wm_side, bm_side, W_side, b_side, W1, b1, g_, be, W2, b2,
             cnt_t, embT_sb, emb_sb, colo, rowo, mode="full"):
    D, M, NB, CH = cfg.D, cfg.M, cfg.NB, cfg.CH
    F32 = mybir.dt.float32
    BF16 = mybir.dt.bfloat16
    if True:
            cur_chunk = [None, -1]
            ntiles_all = int(T.sum())

            def chunk_for(tidx):
                cidx = tidx // CH
                if cur_chunk[1] != cidx:
                    t = io.tile([128, CH, M], F8, tag="vchunk")
                    n = min(CH, ntiles_all - cidx * CH)
                    eng = nc.sync if cidx % 2 == 0 else nc.scalar
                    eng.dma_start(
                        t[:, :n, :],
                        vhw.ap()[cidx][:, :n * M].rearrange(
                            "p (t m) -> p t m", m=M))
                    cur_chunk[0], cur_chunk[1] = t, cidx
                return cur_chunk[0]

            if mode == "empty":
                z = fin.tile([128, 1], F32, tag="z")
                nc.vector.memset(z[:], 0.0)
                return

            if mode == "dma":
                for tidx in range(ntiles_all):
                    chunk_for(tidx)
                return

            tidx = 0
            for s, side in ((0, "col"), (1, "row")):
                out_d = colo if side == "col" else rowo
                for b in range(NB):
                    Tb = int(T[s][b])
                    A_ps = psa.tile([128, M], F32, tag="A")
                    for t in range(Tb):
                        ck = chunk_for(tidx)
                        nc.tensor.matmul(
                            A_ps[:], lhsT=identb[:], rhs=ck[:, tidx % CH, :],
                            start=(t == 0), stop=(t == Tb - 1),
                            skip_group_check=True)
                        tidx += 1

                    # ---- finish for this block of 128 nodes ----
                    n0 = b * 128
                    ET = embT_sb[side][:, n0:n0 + 128]
                    E_t = emb_sb[side][:, b, :]
                    A_t = fin.tile([128, M], F32, tag="A")
                    nc.vector.tensor_copy(A_t[:], A_ps[:])
                    if mode == "main":
                        nc.sync.dma_start(
                            out_d.ap()[n0:n0 + 128, :M], A_t[:])
                        continue

                    # h = emb @ W + b
                    h_ps = ps.tile([128, D], F32, tag="p1")
                    nc.tensor.matmul(h_ps[:], lhsT=ET, rhs=W_side[side][:])
                    h = fin.tile([128, D], F32, tag="h")
                    nc.vector.tensor_add(h[:], h_ps[:], b_side[side][:])

                    # S = A @ Wm + cnt*bm
                    at_ps = ps.tile([M, 128], F32, tag="p2")
                    nc.tensor.transpose(at_ps[:], A_t[:], ident[:])
                    AT = fin.tile([M, 128], F32, tag="AT")
                    nc.vector.tensor_copy(AT[:], at_ps[:])
                    s_ps = ps.tile([128, D], F32, tag="p1")
                    nc.tensor.matmul(s_ps[:], lhsT=AT[:], rhs=Wm_side[side][:])
                    cb = fin.tile([128, D], F32, tag="cb")
                    nc.vector.tensor_scalar_mul(
                        cb[:], in0=bm_side[side][:],
                        scalar1=cnt_t[:, s * NB + b:s * NB + b + 1])
                    S = fin.tile([128, D], F32, tag="S")
                    nc.vector.tensor_add(S[:], s_ps[:], cb[:])

                    # u = h * S
                    u = fin.tile([128, D], F32, tag="u")
                    nc.vector.tensor_mul(u[:], h[:], S[:])

                    # t1 = u @ W1 + b1
                    ut_ps = ps.tile([128, 128], F32, tag="p2")
                    nc.tensor.transpose(ut_ps[:], u[:], ident[:])
                    uT = fin.tile([128, 128], F32, tag="uT")
                    nc.vector.tensor_copy(uT[:], ut_ps[:])
                    t1_ps = ps.tile([128, D], F32, tag="p1")
                    nc.tensor.matmul(t1_ps[:], lhsT=uT[:], rhs=W1[side][:])
                    t1 = fin.tile([128, D], F32, tag="t1")
                    nc.vector.tensor_add(t1[:], t1_ps[:], b1[side][:])

                    # LN(t1) * g + beta
                    stats = fin.tile([128, nc.vector.BN_STATS_DIM], F32, tag="st")
                    nc.vector.bn_stats(stats[:], t1[:])
                    mv = fin.tile([128, nc.vector.BN_AGGR_DIM], F32, tag="mv")
                    nc.vector.bn_aggr(mv[:], stats[:])
                    rstd = fin.tile([128, 1], F32, tag="rs")
                    nc.scalar.activation(
                        rstd[:], mv[:, 1:2],
                        func=mybir.ActivationFunctionType.Sqrt,
                        bias=eps[:], scale=1.0, alpha=0.0)
                    nc.vector.reciprocal(rstd[:], rstd[:])
                    nc.vector.tensor_scalar(
                        t1[:], in0=t1[:], scalar1=mv[:, 0:1], scalar2=rstd[:],
                        op0=mybir.AluOpType.subtract, op1=mybir.AluOpType.mult)
                    nc.vector.tensor_mul(t1[:], t1[:], g_[side][:])
                    nc.vector.tensor_add(t1[:], t1[:], be[side][:])

                    # t2 = ln @ W2 + b2 ; out = emb + t2
                    lt_ps = ps.tile([128, 128], F32, tag="p2")
                    nc.tensor.transpose(lt_ps[:], t1[:], ident[:])
                    lT = fin.tile([128, 128], F32, tag="lT")
                    nc.vector.tensor_copy(lT[:], lt_ps[:])
                    t2_ps = ps.tile([128, D], F32, tag="p1")
                    nc.tensor.matmul(t2_ps[:], lhsT=lT[:], rhs=W2[side][:])
                    ot = fin.tile([128, D], F32, tag="ot")
                    nc.vector.tensor_add(ot[:], t2_ps[:], b2[side][:])
                    nc.vector.tensor_add(ot[:], ot[:], E_t)
                    nc.sync.dma_start(out_d.ap()[n0:n0 + 128, :], ot[:])


def assemble(results, perms, cfg):
    NSH = cfg.NSH
    row = np.empty((cfg.N, cfg.D), np.float32)
    col = np.empty((cfg.N, cfg.D), np.float32)
    for c, r in enumerate(results):
        ord_s2d, ord_d2s = perms[c]
        col[c * NSH + ord_s2d] = r["colo"]
        row[c * NSH + ord_d2s] = r["rowo"]
    return row, col


# ---------------- graded entry point ----------------

_CACHE = {}


def kernel(**inputs):
    cfg = Cfg()
    in_maps, sched, TOT, perms = host_prep(inputs, cfg)
    key = (sched, TOT)
    if key not in _CACHE:
        _CACHE[key] = build_kernel(cfg, sched, TOT)
    nc = _CACHE[key]
    from concourse.bass_utils import run_bass_kernel_spmd
    res = run_bass_kernel_spmd(nc, in_maps, core_ids=list(range(cfg.C)))
    return assemble(res.results, perms, cfg)


# revision 18
# speedup vs baseline: 45.4528x; 1.7344x over previous
"""Trainium2 Bass kernel for nn_NodeEdgeConv (GNN message passing).

Strategy (destination-sharded, matmul segment-sum, batched finish):
- Algebraic reduction: segment_sum(h[idx]*(v@W+b), idx)[n]
    = h[n] * (segment_sum(v, idx)[n] @ W + cnt[n]*b),
  so only the [E, 64] edge payloads need a device-side segment sum; all
  matmuls collapse to node-level GEMMs.
- Edges are sharded by DESTINATION node (node >> 10 -> core), so each core
  computes complete segment sums for its own 1024+1024 node shard; no
  collective needed.
- Host-side degree-sorted slotting: per (core, side), nodes are sorted by
  edge count and assigned to (block, partition) slots; fp8 edge payloads
  are laid out in tiles of [128 tokens, 64] where partition p always
  belongs to node slot p of the current block. The device-side segment sum
  is then just `psum += tile` -- a matmul with a constant fp8 identity
  stationary. Zero per-edge index processing on device.
- Finish (Linear+LayerNorm+Linear residual) runs batched per side in
  transposed orientation [D, 1024]: all weight matmuls use constant
  stationaries (bias/cnt folded in via a 65-row message weight; LayerNorm
  gamma/beta and final bias folded into W2/embeddings on the host). Only
  LayerNorm statistics round-trip through node orientation via PE
  transposes. Outputs are written transposed and unscrambled on the host.
"""

import numpy as np
import ml_dtypes

import concourse.bass as bass
import concourse.bacc as bacc
import concourse.mybir as mybir
import concourse.tile as tile

F32 = mybir.dt.float32
BF16 = mybir.dt.bfloat16
F8 = mybir.dt.float8e4
BF16_NP = ml_dtypes.bfloat16
F8_NP = ml_dtypes.float8_e4m3


class Cfg:
    def __init__(self):
        self.N = 8192          # nodes per side
        self.E = 524288        # edges per type
        self.D = 128
        self.M = 64
        self.C = 8             # cores
        self.NSH = self.N // self.C      # 1024 nodes per core per side
        self.NB = self.NSH // 128        # 8 blocks per side
        self.CH = 192          # tiles per DMA chunk (1.5 MB fp8)


# ---------------- host-side schedule + layout ----------------

def host_prep(inputs, cfg):
    """Shard edges by destination, degree-sort nodes into (block, partition)
    slots, lay out payload tiles, fold biases. Returns
    (in_maps, sched, TOT, perms)."""
    C, NSH, NB, M, CH, D = cfg.C, cfg.NSH, cfg.NB, cfg.M, cfg.CH, cfg.D

    sides = [
        (np.asarray(inputs["e_s2d_dst"]), np.asarray(inputs["v_s2d"], np.float32)),
        (np.asarray(inputs["e_d2s_dst"]), np.asarray(inputs["v_d2s"], np.float32)),
    ]

    percore = [[None] * 2 for _ in range(C)]
    for s, (idx_all, v_all) in enumerate(sides):
        core_of = idx_all // NSH
        for c in range(C):
            esel = np.flatnonzero(core_of == c)
            loc = idx_all[esel] - c * NSH
            cnt = np.bincount(loc, minlength=NSH)
            order = np.argsort(-cnt, kind="stable")
            percore[c][s] = (esel, loc, cnt, order)

    # SPMD envelope: per-block tile count = max over cores of block max count
    T = np.zeros((2, NB), np.int64)
    for s in range(2):
        for c in range(C):
            cnt, order = percore[c][s][2], percore[c][s][3]
            sc = cnt[order]
            for b in range(NB):
                T[s][b] = max(T[s][b], sc[128 * b])
    T = np.maximum(T, 1)
    sched = tuple(int(x) for x in T.reshape(-1))
    block_off = np.zeros((2, NB), np.int64)
    off = 0
    for s in range(2):
        for b in range(NB):
            block_off[s][b] = off
            off += T[s][b]
    TILES = off
    NCHUNK = -(-TILES // CH)
    TOT = NCHUNK * CH

    semb = np.asarray(inputs["src_embed"], np.float32)
    demb = np.asarray(inputs["dst_embed"], np.float32)
    emb_by_side = [demb, semb]     # side 0 (s2d) -> dst nodes, side 1 -> src

    def f32(k):
        return np.asarray(inputs[k], np.float32)

    # side-stacked folded weights (side 0 = "col"/dst, side 1 = "row"/src)
    Wside = np.stack([f32("W_dst"), f32("W_src")]).astype(BF16_NP)
    Wmh = np.stack([
        np.vstack([f32("W_sm"), f32("b_sm")[None]]),
        np.vstack([f32("W_dm"), f32("b_dm")[None]]),
    ]).astype(BF16_NP)                                   # [2, M+1, D]
    W1 = np.stack([f32("col_W1"), f32("row_W1")]).astype(BF16_NP)
    W2g = np.stack([
        f32("col_g")[:, None] * f32("col_W2"),
        f32("row_g")[:, None] * f32("row_W2"),
    ]).astype(BF16_NP)
    b2p = [f32("col_beta") @ f32("col_W2") + f32("col_b2"),
           f32("row_beta") @ f32("row_W2") + f32("row_b2")]
    bcols = np.stack([f32("b_dst"), f32("b_src"),
                      f32("col_b1"), f32("row_b1")], axis=1)   # [128, 4]
    identb = np.eye(128, dtype=F8_NP)
    identb16 = np.eye(128, dtype=BF16_NP)

    common = {"identb": identb, "identb16": identb16, "Wside_b": Wside,
              "Wmh_b": Wmh, "W1_b": W1, "W2g_b": W2g,
              "bcols": np.ascontiguousarray(bcols)}

    in_maps = []
    perms = []
    for c in range(C):
        vtiles = np.zeros((TOT, 128, M), F8_NP)
        cntR = np.zeros((2, 1, NB, 128), BF16_NP)
        embT16 = np.zeros((2, D, NSH), BF16_NP)
        embTb2 = np.zeros((2, D, NSH), np.float32)
        ords = []
        for s, (idx_all, v_all) in enumerate(sides):
            esel, loc, cnt, order = percore[c][s]
            ords.append(order)
            pos = np.empty(NSH, np.int64)
            pos[order] = np.arange(NSH)
            eorder = np.argsort(loc, kind="stable")
            starts = np.zeros(NSH + 1, np.int64)
            np.cumsum(cnt, out=starts[1:])
            rank = np.arange(len(eorder)) - starts[loc[eorder]]
            p_of = pos[loc[eorder]]
            tile_of = block_off[s][p_of // 128] + rank
            flat = tile_of * 128 + (p_of % 128)
            vtiles.reshape(-1, M)[flat] = v_all[esel[eorder]].astype(F8_NP)
            cntR[s, 0] = cnt[order].astype(BF16_NP).reshape(NB, 128)
            embT = emb_by_side[s][c * NSH:(c + 1) * NSH][order].T
            embT16[s] = embT.astype(BF16_NP)
            embTb2[s] = embT + b2p[s][:, None]
        m = dict(common)
        m["vhw"] = np.ascontiguousarray(
            vtiles.reshape(NCHUNK, CH, 128, M).transpose(0, 2, 1, 3)
            .reshape(NCHUNK, 128, CH * M))
        m["cntR"] = cntR
        m["embT16"] = embT16
        m["embTb2"] = embTb2
        in_maps.append(m)
        perms.append(ords)
    return in_maps, sched, TOT, perms


# ---------------- device kernel ----------------

def build_kernel(cfg, sched, TOT, reps=1, mode="full"):
    import contextlib
    C, D, M, NSH, NB, CH = cfg.C, cfg.D, cfg.M, cfg.NSH, cfg.NB, cfg.CH
    T = np.asarray(sched, np.int64).reshape(2, NB)
    NCHUNK = TOT // CH
    nc = bacc.Bacc("TRN2", target_bir_lowering=False, debug=False, num_devices=C)

    vhw = nc.dram_tensor("vhw", [NCHUNK, 128, CH * M], F8, kind="ExternalInput")
    identb_d = nc.dram_tensor("identb", [128, 128], F8, kind="ExternalInput")
    identb16_d = nc.dram_tensor("identb16", [128, 128], BF16, kind="ExternalInput")
    Wside_d = nc.dram_tensor("Wside_b", [2, D, D], BF16, kind="ExternalInput")
    Wmh_d = nc.dram_tensor("Wmh_b", [2, M + 1, D], BF16, kind="ExternalInput")
    W1_d = nc.dram_tensor("W1_b", [2, D, D], BF16, kind="ExternalInput")
    W2g_d = nc.dram_tensor("W2g_b", [2, D, D], BF16, kind="ExternalInput")
    bcols_d = nc.dram_tensor("bcols", [128, 4], F32, kind="ExternalInput")
    cntR_d = nc.dram_tensor("cntR", [2, 1, NB, 128], BF16, kind="ExternalInput")
    embT16_d = nc.dram_tensor("embT16", [2, D, NSH], BF16, kind="ExternalInput")
    embTb2_d = nc.dram_tensor("embTb2", [2, D, NSH], F32, kind="ExternalInput")
    rowo = nc.dram_tensor("rowo", [D, NSH], F32, kind="ExternalOutput")
    colo = nc.dram_tensor("colo", [D, NSH], F32, kind="ExternalOutput")

    with tile.TileContext(nc) as tc:
        with (
            tc.tile_pool(name="const", bufs=1) as const,
            tc.tile_pool(name="io", bufs=3) as io,
            tc.tile_pool(name="fin", bufs=2) as fin,
            tc.tile_pool(name="psa", bufs=2, space="PSUM") as psa,
            tc.tile_pool(name="psb", bufs=1, space="PSUM") as psb,
            tc.tile_pool(name="psw", bufs=1, space="PSUM") as psw,
        ):
            identb = const.tile([128, 128], F8)
            nc.sync.dma_start(identb[:], identb_d.ap())
            identb16 = const.tile([128, 128], BF16)
            nc.sync.dma_start(identb16[:], identb16_d.ap())
            eps = const.tile([128, 1], F32)
            nc.vector.memset(eps[:], 1e-5)

            def load2(dram, shp, tag):
                ts = []
                for s in range(2):
                    t = const.tile(shp, BF16, tag=f"{tag}{s}")
                    nc.sync.dma_start(t[:], dram.ap()[s])
                    ts.append(t)
                return ts

            Wside_sb = load2(Wside_d, [D, D], "Wside")
            Wmh_sb = load2(Wmh_d, [M + 1, D], "Wmh")
            W1_sb = load2(W1_d, [D, D], "W1")
            W2g_sb = load2(W2g_d, [D, D], "W2g")
            bcols = const.tile([128, 4], F32)
            nc.sync.dma_start(bcols[:], bcols_d.ap())
            embT16_sb = const.tile([128, 2, NSH], BF16)
            nc.sync.dma_start(embT16_sb[:], embT16_d.ap().rearrange(
                "s d n -> d s n"))
            embTb2_sb = const.tile([128, 2, NSH], F32)
            nc.sync.dma_start(embTb2_sb[:], embTb2_d.ap().rearrange(
                "s d n -> d s n"))
            # AT tiles: rows 0..63 written per side per rep; row 64 = cnt
            AT_sb = []
            for s in range(2):
                t = const.tile([M + 1, NB, 128], BF16, tag=f"AT{s}")
                nc.sync.dma_start(t[M:M + 1, :, :], cntR_d.ap()[s])
                AT_sb.append(t)

            rep_ctx = tc.For_i(0, reps) if reps > 1 else contextlib.nullcontext()
            with rep_ctx:
                run_body(nc, tc, cfg, T, io, fin, psa, psb, psw,
                         vhw, identb, identb16, eps, Wside_sb, Wmh_sb,
                         W1_sb, W2g_sb, bcols, embT16_sb, embTb2_sb,
                         AT_sb, colo, rowo, mode)

    nc.compile()
    return nc


def run_body(nc, tc, cfg, T, io, fin, psa, psb, psw, vhw, identb, identb16,
             eps, Wside_sb, Wmh_sb, W1_sb, W2g_sb, bcols, embT16_sb,
             embTb2_sb, AT_sb, colo, rowo, mode="full"):
    D, M, NB, CH = cfg.D, cfg.M, cfg.NB, cfg.CH
    NW = NB * 128          # nodes per side (1024)
    cur_chunk = [None, -1]
    ntiles_all = int(T.sum())

    def chunk_for(tidx):
        cidx = tidx // CH
        if cur_chunk[1] != cidx:
            t = io.tile([128, CH, M], F8, tag="vchunk")
            n = min(CH, ntiles_all - cidx * CH)
            nc.sync.dma_start(
                t[:, :n, :],
                vhw.ap()[cidx][:, :n * M].rearrange("p (t m) -> p t m", m=M))
            cur_chunk[0], cur_chunk[1] = t, cidx
        return cur_chunk[0]

    if mode == "empty":
        z = fin.tile([128, 1], F32, tag="z")
        nc.vector.memset(z[:], 0.0)
        return

    if mode == "dma":
        for tidx in range(ntiles_all):
            chunk_for(tidx)
        return

    tidx = 0
    for s in range(2):
        out_d = colo if s == 0 else rowo

        # ---- segment sums for all 8 blocks into one PSUM bank ----
        A_w = psa.tile([128, NB * M], F32, tag="A")
        for b in range(NB):
            Tb = int(T[s][b])
            for t in range(Tb):
                ck = chunk_for(tidx)
                nc.tensor.matmul(
                    A_w[:, b * M:(b + 1) * M], lhsT=identb[:],
                    rhs=ck[:, tidx % CH, :],
                    start=(t == 0), stop=(t == Tb - 1),
                    skip_group_check=True)
                tidx += 1

        A_sb = fin.tile([128, NB, M], BF16, tag="Asb")
        nc.vector.tensor_copy(
            A_sb[:].rearrange("p b m -> p (b m)"), A_w[:])

        if mode == "main":
            af = fin.tile([128, NB * M], F32, tag="Af")
            nc.vector.tensor_copy(af[:], A_w[:])
            nc.sync.dma_start(out_d.ap()[:, :NB * M // 2],
                              af[:, :NB * M // 2])
            continue

        # ---- A^T via PE transposes -> AT rows 0..63 (row 64 = cnt) ----
        at_w = psb.tile([M, NB * 128], BF16, tag="tr1")
        for b in range(NB):
            nc.tensor.transpose(
                at_w[:, b * 128:(b + 1) * 128], A_sb[:, b, :], identb16[:])
        nc.vector.tensor_copy(
            AT_sb[s][:M, :, :].rearrange("m b n -> m (b n)"), at_w[:])

        # ---- h^T = (emb @ Wside)^T ; S^T = (A @ Wm + cnt*bm)^T ----
        h_ps = psw.tile([128, 2, 512], F32, tag="mm1")
        s_ps = psw.tile([128, 2, 512], F32, tag="mm2")
        for j in range(2):
            nc.tensor.matmul(
                h_ps[:, j, :], lhsT=Wside_sb[s][:],
                rhs=embT16_sb[:, s, j * 512:(j + 1) * 512])
            nc.tensor.matmul(
                s_ps[:, j, :], lhsT=Wmh_sb[s][:],
                rhs=AT_sb[s][:].rearrange("m b n -> m (b n)")[
                    :, j * 512:(j + 1) * 512])
        h2 = fin.tile([128, NW], F32, tag="h2")
        nc.vector.tensor_scalar_add(
            h2[:], h_ps[:].rearrange("p j n -> p (j n)"),
            scalar1=bcols[:, s:s + 1])
        u_sb = fin.tile([128, NW], BF16, tag="u")
        nc.vector.tensor_mul(
            u_sb[:], h2[:], s_ps[:].rearrange("p j n -> p (j n)"))

        # ---- t1^T = (u @ W1)^T + b1 ----
        t1_ps = psw.tile([128, 2, 512], F32, tag="mm1")
        for j in range(2):
            nc.tensor.matmul(t1_ps[:, j, :], lhsT=W1_sb[s][:],
                             rhs=u_sb[:, j * 512:(j + 1) * 512])
        t1T = fin.tile([128, NW], BF16, tag="t1T")
        nc.vector.tensor_scalar_add(
            t1T[:], t1_ps[:].rearrange("p j n -> p (j n)"),
            scalar1=bcols[:, 2 + s:3 + s])

        # ---- LayerNorm stats in node orientation ----
        t1w = psb.tile([128, NW], BF16, tag="tr2")
        for b in range(NB):
            nc.tensor.transpose(
                t1w[:, b * 128:(b + 1) * 128],
                t1T[:, b * 128:(b + 1) * 128], identb16[:])
        t1n = fin.tile([128, NB, 128], BF16, tag="t1n")
        nc.vector.tensor_copy(
            t1n[:].rearrange("p b d -> p (b d)"), t1w[:])
        sum_t = fin.tile([128, NB], F32, tag="sum")
        nc.vector.tensor_reduce(sum_t[:], t1n[:], axis=mybir.AxisListType.X,
                                op=mybir.AluOpType.add)
        sq = fin.tile([128, NB, 128], BF16, tag="sq")
        nc.scalar.activation(sq[:], t1n[:],
                             func=mybir.ActivationFunctionType.Square)
        ssq = fin.tile([128, NB], F32, tag="ssq")
        nc.vector.tensor_reduce(ssq[:], sq[:], axis=mybir.AxisListType.X,
                                op=mybir.AluOpType.add)
        mu = fin.tile([128, NB], F32, tag="mu")
        nc.vector.tensor_scalar_mul(mu[:], in0=sum_t[:], scalar1=1.0 / D)
        var = fin.tile([128, NB], F32, tag="var")
        nc.vector.tensor_scalar_mul(var[:], in0=ssq[:], scalar1=1.0 / D)
        m2 = fin.tile([128, NB], F32, tag="m2")
        nc.vector.tensor_mul(m2[:], mu[:], mu[:])
        nc.vector.tensor_sub(var[:], var[:], m2[:])
        rstd = fin.tile([128, NB], F32, tag="rstd")
        nc.scalar.activation(rstd[:], var[:],
                             func=mybir.ActivationFunctionType.Sqrt,
                             bias=eps[:], scale=1.0)
        nc.vector.reciprocal(rstd[:], rstd[:])
        that = fin.tile([128, NB, 128], BF16, tag="that")
        for b in range(NB):
            nc.vector.tensor_scalar(
                that[:, b, :], in0=t1n[:, b, :],
                scalar1=mu[:, b:b + 1], scalar2=rstd[:, b:b + 1],
                op0=mybir.AluOpType.subtract, op1=mybir.AluOpType.mult)

        # ---- t2^T = (that @ W2g)^T ; out^T = t2^T + embT + b2p ----
        tt_w = psb.tile([128, NW], BF16, tag="tr2")
        for b in range(NB):
            nc.tensor.transpose(
                tt_w[:, b * 128:(b + 1) * 128], that[:, b, :], identb16[:])
        tT = fin.tile([128, NW], BF16, tag="tT")
        nc.vector.tensor_copy(tT[:], tt_w[:])
        t2_ps = psw.tile([128, 2, 512], F32, tag="mm2")
        for j in range(2):
            nc.tensor.matmul(t2_ps[:, j, :], lhsT=W2g_sb[s][:],
                             rhs=tT[:, j * 512:(j + 1) * 512])
        ot = fin.tile([128, NW], F32, tag="ot")
        nc.vector.tensor_add(
            ot[:], t2_ps[:].rearrange("p j n -> p (j n)"),
            embTb2_sb[:, s, :])
        nc.sync.dma_start(out_d.ap(), ot[:])


def assemble(results, perms, cfg):
    NSH = cfg.NSH
    row = np.empty((cfg.N, cfg.D), np.float32)
    col = np.empty((cfg.N, cfg.D), np.float32)
    for c, r in enumerate(results):
        ord_s2d, ord_d2s = perms[c]
        col[c * NSH + ord_s2d] = r["colo"].T
        row[c * NSH + ord_d2s] = r["rowo"].T
    return row, col


# ---------------- graded entry point ----------------

_CACHE = {}


def kernel(**inputs):
    cfg = Cfg()
    in_maps, sched, TOT, perms = host_prep(inputs, cfg)
    key = (sched, TOT)
    if key not in _CACHE:
        _CACHE[key] = build_kernel(cfg, sched, TOT)
    nc = _CACHE[key]
    from concourse.bass_utils import run_bass_kernel_spmd
    res = run_bass_kernel_spmd(nc, in_maps, core_ids=list(range(cfg.C)))
    return assemble(res.results, perms, cfg)


# revision 20
# speedup vs baseline: 50.7358x; 1.1162x over previous
"""Trainium2 Bass kernel for nn_NodeEdgeConv (GNN message passing).

Strategy (destination-sharded, matmul segment-sum, batched finish):
- Algebraic reduction: segment_sum(h[idx]*(v@W+b), idx)[n]
    = h[n] * (segment_sum(v, idx)[n] @ W + cnt[n]*b),
  so only the [E, 64] edge payloads need a device-side segment sum; all
  matmuls collapse to node-level GEMMs.
- Edges are sharded by DESTINATION node (node >> 10 -> core), so each core
  computes complete segment sums for its own 1024+1024 node shard; no
  collective needed.
- Host-side degree-sorted slotting: per (core, side), nodes are sorted by
  edge count and assigned to (block, partition) slots; fp8 edge payloads
  are laid out in tiles of [128 tokens, 64] where partition p always
  belongs to node slot p of the current block. The device-side segment sum
  is then just `psum += tile` -- a matmul with a constant fp8 identity
  stationary. Zero per-edge index processing on device.
- Finish (Linear+LayerNorm+Linear residual) runs batched per side in
  transposed orientation [D, 1024]: all weight matmuls use constant
  stationaries (bias/cnt folded in via a 65-row message weight; LayerNorm
  gamma/beta and final bias folded into W2/embeddings on the host). Only
  LayerNorm statistics round-trip through node orientation via PE
  transposes. Outputs are written transposed and unscrambled on the host.
"""

import numpy as np
import ml_dtypes

import concourse.bass as bass
import concourse.bacc as bacc
import concourse.mybir as mybir
import concourse.tile as tile

F32 = mybir.dt.float32
BF16 = mybir.dt.bfloat16
F8 = mybir.dt.float8e4
BF16_NP = ml_dtypes.bfloat16
F8_NP = ml_dtypes.float8_e4m3


class Cfg:
    def __init__(self):
        self.N = 8192          # nodes per side
        self.E = 524288        # edges per type
        self.D = 128
        self.M = 64
        self.C = 8             # cores
        self.NSH = self.N // self.C      # 1024 nodes per core per side
        self.NB = self.NSH // 128        # 8 blocks per side
        self.CH = 192          # tiles per DMA chunk (1.5 MB fp8)


# ---------------- host-side schedule + layout ----------------

def host_prep(inputs, cfg):
    """Shard edges by destination, degree-sort nodes into (block, partition)
    slots, lay out payload tiles, fold biases. Returns
    (in_maps, sched, TOT, perms)."""
    C, NSH, NB, M, CH, D = cfg.C, cfg.NSH, cfg.NB, cfg.M, cfg.CH, cfg.D

    sides = [
        (np.asarray(inputs["e_s2d_dst"]), np.asarray(inputs["v_s2d"], np.float32)),
        (np.asarray(inputs["e_d2s_dst"]), np.asarray(inputs["v_d2s"], np.float32)),
    ]

    percore = [[None] * 2 for _ in range(C)]
    for s, (idx_all, v_all) in enumerate(sides):
        core_of = idx_all // NSH
        for c in range(C):
            esel = np.flatnonzero(core_of == c)
            loc = idx_all[esel] - c * NSH
            cnt = np.bincount(loc, minlength=NSH)
            order = np.argsort(-cnt, kind="stable")
            percore[c][s] = (esel, loc, cnt, order)

    # SPMD envelope: per-block tile count = max over cores of block max count
    T = np.zeros((2, NB), np.int64)
    for s in range(2):
        for c in range(C):
            cnt, order = percore[c][s][2], percore[c][s][3]
            sc = cnt[order]
            for b in range(NB):
                T[s][b] = max(T[s][b], sc[128 * b])
    T = np.maximum(T, 1)
    sched = tuple(int(x) for x in T.reshape(-1))
    block_off = np.zeros((2, NB), np.int64)
    off = 0
    for s in range(2):
        for b in range(NB):
            block_off[s][b] = off
            off += T[s][b]
    TILES = off
    NCHUNK = -(-TILES // CH)
    TOT = NCHUNK * CH

    semb = np.asarray(inputs["src_embed"], np.float32)
    demb = np.asarray(inputs["dst_embed"], np.float32)
    emb_by_side = [demb, semb]     # side 0 (s2d) -> dst nodes, side 1 -> src

    def f32(k):
        return np.asarray(inputs[k], np.float32)

    # side-stacked folded weights (side 0 = "col"/dst, side 1 = "row"/src)
    Wside = np.stack([f32("W_dst"), f32("W_src")]).astype(BF16_NP)
    Wmh = np.stack([
        np.vstack([f32("W_sm"), f32("b_sm")[None]]),
        np.vstack([f32("W_dm"), f32("b_dm")[None]]),
    ]).astype(BF16_NP)                                   # [2, M+1, D]
    W1 = np.stack([f32("col_W1"), f32("row_W1")]).astype(BF16_NP)
    W2g = np.stack([
        f32("col_g")[:, None] * f32("col_W2"),
        f32("row_g")[:, None] * f32("row_W2"),
    ]).astype(BF16_NP)
    b2p = [f32("col_beta") @ f32("col_W2") + f32("col_b2"),
           f32("row_beta") @ f32("row_W2") + f32("row_b2")]
    bcols = np.stack([f32("b_dst"), f32("b_src"),
                      f32("col_b1"), f32("row_b1")], axis=1)   # [128, 4]
    identb = np.eye(128, dtype=F8_NP)
    identb16 = np.eye(128, dtype=BF16_NP)

    common = {"identb": identb, "identb16": identb16, "Wside_b": Wside,
              "Wmh_b": Wmh, "W1_b": W1, "W2g_b": W2g,
              "bcols": np.ascontiguousarray(bcols)}

    in_maps = []
    perms = []
    for c in range(C):
        vtiles = np.zeros((TOT, 128, M), F8_NP)
        cntR = np.zeros((2, 1, NB, 128), BF16_NP)
        embT16 = np.zeros((2, D, NSH), BF16_NP)
        embTb2 = np.zeros((2, D, NSH), np.float32)
        ords = []
        for s, (idx_all, v_all) in enumerate(sides):
            esel, loc, cnt, order = percore[c][s]
            ords.append(order)
            pos = np.empty(NSH, np.int64)
            pos[order] = np.arange(NSH)
            eorder = np.argsort(loc, kind="stable")
            starts = np.zeros(NSH + 1, np.int64)
            np.cumsum(cnt, out=starts[1:])
            rank = np.arange(len(eorder)) - starts[loc[eorder]]
            p_of = pos[loc[eorder]]
            tile_of = block_off[s][p_of // 128] + rank
            flat = tile_of * 128 + (p_of % 128)
            vtiles.reshape(-1, M)[flat] = v_all[esel[eorder]].astype(F8_NP)
            cntR[s, 0] = cnt[order].astype(BF16_NP).reshape(NB, 128)
            embT = emb_by_side[s][c * NSH:(c + 1) * NSH][order].T
            embT16[s] = embT.astype(BF16_NP)
            embTb2[s] = embT + b2p[s][:, None]
        m = dict(common)
        m["vhw"] = np.ascontiguousarray(
            vtiles.reshape(NCHUNK, CH, 128, M).transpose(0, 2, 1, 3)
            .reshape(NCHUNK, 128, CH * M))
        m["cntR"] = cntR
        m["embT16"] = embT16
        m["embTb2"] = embTb2
        in_maps.append(m)
        perms.append(ords)
    return in_maps, sched, TOT, perms


# ---------------- device kernel ----------------

def build_kernel(cfg, sched, TOT, reps=1, mode="full"):
    import contextlib
    C, D, M, NSH, NB, CH = cfg.C, cfg.D, cfg.M, cfg.NSH, cfg.NB, cfg.CH
    T = np.asarray(sched, np.int64).reshape(2, NB)
    NCHUNK = TOT // CH
    nc = bacc.Bacc("TRN2", target_bir_lowering=False, debug=False, num_devices=C)

    vhw = nc.dram_tensor("vhw", [NCHUNK, 128, CH * M], F8, kind="ExternalInput")
    identb_d = nc.dram_tensor("identb", [128, 128], F8, kind="ExternalInput")
    identb16_d = nc.dram_tensor("identb16", [128, 128], BF16, kind="ExternalInput")
    Wside_d = nc.dram_tensor("Wside_b", [2, D, D], BF16, kind="ExternalInput")
    Wmh_d = nc.dram_tensor("Wmh_b", [2, M + 1, D], BF16, kind="ExternalInput")
    W1_d = nc.dram_tensor("W1_b", [2, D, D], BF16, kind="ExternalInput")
    W2g_d = nc.dram_tensor("W2g_b", [2, D, D], BF16, kind="ExternalInput")
    bcols_d = nc.dram_tensor("bcols", [128, 4], F32, kind="ExternalInput")
    cntR_d = nc.dram_tensor("cntR", [2, 1, NB, 128], BF16, kind="ExternalInput")
    embT16_d = nc.dram_tensor("embT16", [2, D, NSH], BF16, kind="ExternalInput")
    embTb2_d = nc.dram_tensor("embTb2", [2, D, NSH], F32, kind="ExternalInput")
    rowo = nc.dram_tensor("rowo", [D, NSH], F32, kind="ExternalOutput")
    colo = nc.dram_tensor("colo", [D, NSH], F32, kind="ExternalOutput")

    with tile.TileContext(nc) as tc:
        with (
            tc.tile_pool(name="const", bufs=1) as const,
            tc.tile_pool(name="io", bufs=3) as io,
            tc.tile_pool(name="fin", bufs=2) as fin,
            tc.tile_pool(name="psa", bufs=2, space="PSUM") as psa,
            tc.tile_pool(name="psb", bufs=1, space="PSUM") as psb,
            tc.tile_pool(name="psw", bufs=1, space="PSUM") as psw,
        ):
            identb = const.tile([128, 128], F8)
            nc.sync.dma_start(identb[:], identb_d.ap())
            identb16 = const.tile([128, 128], BF16)
            nc.sync.dma_start(identb16[:], identb16_d.ap())
            eps = const.tile([128, 1], F32)
            nc.vector.memset(eps[:], 1e-5)

            def load2(dram, shp, tag):
                ts = []
                for s in range(2):
                    t = const.tile(shp, BF16, tag=f"{tag}{s}")
                    nc.sync.dma_start(t[:], dram.ap()[s])
                    ts.append(t)
                return ts

            Wside_sb = load2(Wside_d, [D, D], "Wside")
            Wmh_sb = load2(Wmh_d, [M + 1, D], "Wmh")
            W1_sb = load2(W1_d, [D, D], "W1")
            W2g_sb = load2(W2g_d, [D, D], "W2g")
            bcols = const.tile([128, 4], F32)
            nc.sync.dma_start(bcols[:], bcols_d.ap())
            embT16_sb = const.tile([128, 2, NSH], BF16)
            nc.sync.dma_start(embT16_sb[:], embT16_d.ap().rearrange(
                "s d n -> d s n"))
            embTb2_sb = const.tile([128, 2, NSH], F32)
            nc.sync.dma_start(embTb2_sb[:], embTb2_d.ap().rearrange(
                "s d n -> d s n"))
            # AT tiles: rows 0..63 written per side per rep; row 64 = cnt
            AT_sb = []
            for s in range(2):
                t = const.tile([M + 1, NB, 128], BF16, tag=f"AT{s}")
                nc.sync.dma_start(t[M:M + 1, :, :], cntR_d.ap()[s])
                AT_sb.append(t)

            rep_ctx = tc.For_i(0, reps) if reps > 1 else contextlib.nullcontext()
            with rep_ctx:
                run_body(nc, tc, cfg, T, io, fin, psa, psb, psw,
                         vhw, identb, identb16, eps, Wside_sb, Wmh_sb,
                         W1_sb, W2g_sb, bcols, embT16_sb, embTb2_sb,
                         AT_sb, colo, rowo, mode)

    nc.compile()
    return nc


def run_body(nc, tc, cfg, T, io, fin, psa, psb, psw, vhw, identb, identb16,
             eps, Wside_sb, Wmh_sb, W1_sb, W2g_sb, bcols, embT16_sb,
             embTb2_sb, AT_sb, colo, rowo, mode="full"):
    D, M, NB, CH = cfg.D, cfg.M, cfg.NB, cfg.CH
    NW = NB * 128          # nodes per side (1024)
    cur_chunk = [None, -1]
    ntiles_all = int(T.sum())

    def chunk_for(tidx):
        cidx = tidx // CH
        if cur_chunk[1] != cidx:
            t = io.tile([128, CH, M], F8, tag="vchunk")
            n = min(CH, ntiles_all - cidx * CH)
            nc.sync.dma_start(
                t[:, :n, :],
                vhw.ap()[cidx][:, :n * M].rearrange("p (t m) -> p t m", m=M))
            cur_chunk[0], cur_chunk[1] = t, cidx
        return cur_chunk[0]

    if mode == "empty":
        z = fin.tile([128, 1], F32, tag="z")
        nc.vector.memset(z[:], 0.0)
        return

    if mode == "dma":
        for tidx in range(ntiles_all):
            chunk_for(tidx)
        return

    tidx = 0
    for s in range(2):
        out_d = colo if s == 0 else rowo

        # ---- segment sums for all 8 blocks into one PSUM bank ----
        A_w = psa.tile([128, NB * M], F32, tag="A")
        for b in range(NB):
            Tb = int(T[s][b])
            for t in range(Tb):
                ck = chunk_for(tidx)
                nc.tensor.matmul(
                    A_w[:, b * M:(b + 1) * M], lhsT=identb[:],
                    rhs=ck[:, tidx % CH, :],
                    start=(t == 0), stop=(t == Tb - 1),
                    skip_group_check=True)
                tidx += 1

        A_sb = fin.tile([128, NB, M], BF16, tag="Asb")
        nc.vector.tensor_copy(
            A_sb[:].rearrange("p b m -> p (b m)"), A_w[:])

        if mode == "main":
            af = fin.tile([128, NB * M], F32, tag="Af")
            nc.vector.tensor_copy(af[:], A_w[:])
            nc.sync.dma_start(out_d.ap()[:, :NB * M // 2],
                              af[:, :NB * M // 2])
            continue

        # ---- A^T via PE transposes -> AT rows 0..63 (row 64 = cnt) ----
        at_w = psb.tile([M, NB * 128], BF16, tag="tr1")
        for b in range(NB):
            nc.tensor.transpose(
                at_w[:, b * 128:(b + 1) * 128], A_sb[:, b, :], identb16[:])
        nc.scalar.activation(
            AT_sb[s][:M, :, :].rearrange("m b n -> m (b n)"), at_w[:],
            func=mybir.ActivationFunctionType.Copy)

        # ---- h^T = (emb @ Wside)^T ; S^T = (A @ Wm + cnt*bm)^T ----
        h_ps = psw.tile([128, 2, 512], F32, tag="mm1")
        s_ps = psw.tile([128, 2, 512], F32, tag="mm2")
        for j in range(2):
            nc.tensor.matmul(
                h_ps[:, j, :], lhsT=Wside_sb[s][:],
                rhs=embT16_sb[:, s, j * 512:(j + 1) * 512])
            nc.tensor.matmul(
                s_ps[:, j, :], lhsT=Wmh_sb[s][:],
                rhs=AT_sb[s][:].rearrange("m b n -> m (b n)")[
                    :, j * 512:(j + 1) * 512])
        h2 = fin.tile([128, NW], F32, tag="h2")
        nc.vector.tensor_scalar_add(
            h2[:], h_ps[:].rearrange("p j n -> p (j n)"),
            scalar1=bcols[:, s:s + 1])
        u_sb = fin.tile([128, NW], BF16, tag="u")
        nc.vector.tensor_mul(
            u_sb[:], h2[:], s_ps[:].rearrange("p j n -> p (j n)"))

        # ---- t1^T = (u @ W1)^T + b1 ----
        t1_ps = psw.tile([128, 2, 512], F32, tag="mm1")
        for j in range(2):
            nc.tensor.matmul(t1_ps[:, j, :], lhsT=W1_sb[s][:],
                             rhs=u_sb[:, j * 512:(j + 1) * 512])
        t1T = fin.tile([128, NW], BF16, tag="t1T")
        nc.vector.tensor_scalar_add(
            t1T[:], t1_ps[:].rearrange("p j n -> p (j n)"),
            scalar1=bcols[:, 2 + s:3 + s])

        # ---- LayerNorm stats in node orientation ----
        t1w = psb.tile([128, NW], BF16, tag="tr2")
        for b in range(NB):
            nc.tensor.transpose(
                t1w[:, b * 128:(b + 1) * 128],
                t1T[:, b * 128:(b + 1) * 128], identb16[:])
        t1n = fin.tile([128, NB, 128], BF16, tag="t1n")
        nc.scalar.activation(
            t1n[:].rearrange("p b d -> p (b d)"), t1w[:],
            func=mybir.ActivationFunctionType.Copy)
        sum_t = fin.tile([128, NB], F32, tag="sum")
        nc.vector.tensor_reduce(sum_t[:], t1n[:], axis=mybir.AxisListType.X,
                                op=mybir.AluOpType.add)
        sq = fin.tile([128, NB, 128], BF16, tag="sq")
        nc.scalar.activation(sq[:], t1n[:],
                             func=mybir.ActivationFunctionType.Square)
        ssq = fin.tile([128, NB], F32, tag="ssq")
        nc.vector.tensor_reduce(ssq[:], sq[:], axis=mybir.AxisListType.X,
                                op=mybir.AluOpType.add)
        mu = fin.tile([128, NB], F32, tag="mu")
        nc.vector.tensor_scalar_mul(mu[:], in0=sum_t[:], scalar1=1.0 / D)
        var = fin.tile([128, NB], F32, tag="var")
        nc.vector.tensor_scalar_mul(var[:], in0=ssq[:], scalar1=1.0 / D)
        m2 = fin.tile([128, NB], F32, tag="m2")
        nc.vector.tensor_mul(m2[:], mu[:], mu[:])
        nc.vector.tensor_sub(var[:], var[:], m2[:])
        rstd = fin.tile([128, NB], F32, tag="rstd")
        nc.scalar.activation(rstd[:], var[:],
                             func=mybir.ActivationFunctionType.Sqrt,
                             bias=eps[:], scale=1.0)
        nc.vector.reciprocal(rstd[:], rstd[:])
        that = fin.tile([128, NB, 128], BF16, tag="that")
        for b in range(NB):
            nc.vector.tensor_scalar(
                that[:, b, :], in0=t1n[:, b, :],
                scalar1=mu[:, b:b + 1], scalar2=rstd[:, b:b + 1],
                op0=mybir.AluOpType.subtract, op1=mybir.AluOpType.mult)

        # ---- t2^T = (that @ W2g)^T ; out^T = t2^T + embT + b2p ----
        tt_w = psb.tile([128, NW], BF16, tag="tr2")
        for b in range(NB):
            nc.tensor.transpose(
                tt_w[:, b * 128:(b + 1) * 128], that[:, b, :], identb16[:])
        tT = fin.tile([128, NW], BF16, tag="tT")
        nc.scalar.activation(tT[:], tt_w[:],
                             func=mybir.ActivationFunctionType.Copy)
        t2_ps = psw.tile([128, 2, 512], F32, tag="mm2")
        for j in range(2):
            nc.tensor.matmul(t2_ps[:, j, :], lhsT=W2g_sb[s][:],
                             rhs=tT[:, j * 512:(j + 1) * 512])
        ot = fin.tile([128, NW], F32, tag="ot")
        nc.vector.tensor_add(
            ot[:], t2_ps[:].rearrange("p j n -> p (j n)"),
            embTb2_sb[:, s, :])
        nc.sync.dma_start(out_d.ap(), ot[:])


def assemble(results, perms, cfg):
    NSH = cfg.NSH
    row = np.empty((cfg.N, cfg.D), np.float32)
    col = np.empty((cfg.N, cfg.D), np.float32)
    for c, r in enumerate(results):
        ord_s2d, ord_d2s = perms[c]
        col[c * NSH + ord_s2d] = r["colo"].T
        row[c * NSH + ord_d2s] = r["rowo"].T
    return row, col


# ---------------- graded entry point ----------------

_CACHE = {}


def kernel(**inputs):
    cfg = Cfg()
    in_maps, sched, TOT, perms = host_prep(inputs, cfg)
    key = (sched, TOT)
    if key not in _CACHE:
        _CACHE[key] = build_kernel(cfg, sched, TOT)
    nc = _CACHE[key]
    from concourse.bass_utils import run_bass_kernel_spmd
    res = run_bass_kernel_spmd(nc, in_maps, core_ids=list(range(cfg.C)))
    return assemble(res.results, perms, cfg)


# revision 21
# speedup vs baseline: 51.4502x; 1.0141x over previous
"""Trainium2 Bass kernel for nn_NodeEdgeConv (GNN message passing).

Strategy (destination-sharded, matmul segment-sum, batched finish):
- Algebraic reduction: segment_sum(h[idx]*(v@W+b), idx)[n]
    = h[n] * (segment_sum(v, idx)[n] @ W + cnt[n]*b),
  so only the [E, 64] edge payloads need a device-side segment sum; all
  matmuls collapse to node-level GEMMs.
- Edges are sharded by DESTINATION node (node >> 10 -> core), so each core
  computes complete segment sums for its own 1024+1024 node shard; no
  collective needed.
- Host-side degree-sorted slotting: per (core, side), nodes are sorted by
  edge count and assigned to (block, partition) slots; fp8 edge payloads
  are laid out in tiles of [128 tokens, 64] where partition p always
  belongs to node slot p of the current block. The device-side segment sum
  is then just `psum += tile` -- a matmul with a constant fp8 identity
  stationary. Zero per-edge index processing on device.
- Finish (Linear+LayerNorm+Linear residual) runs batched per side in
  transposed orientation [D, 1024]: all weight matmuls use constant
  stationaries (bias/cnt folded in via a 65-row message weight; LayerNorm
  gamma/beta and final bias folded into W2/embeddings on the host). Only
  LayerNorm statistics round-trip through node orientation via PE
  transposes. Outputs are written transposed and unscrambled on the host.
"""

import numpy as np
import ml_dtypes

import concourse.bass as bass
import concourse.bacc as bacc
import concourse.mybir as mybir
import concourse.tile as tile

F32 = mybir.dt.float32
BF16 = mybir.dt.bfloat16
F8 = mybir.dt.float8e4
BF16_NP = ml_dtypes.bfloat16
F8_NP = ml_dtypes.float8_e4m3


class Cfg:
    def __init__(self):
        self.N = 8192          # nodes per side
        self.E = 524288        # edges per type
        self.D = 128
        self.M = 64
        self.C = 8             # cores
        self.NSH = self.N // self.C      # 1024 nodes per core per side
        self.NB = self.NSH // 128        # 8 blocks per side
        self.CH = 384          # tiles per DMA chunk (3 MB fp8)


# ---------------- host-side schedule + layout ----------------

def host_prep(inputs, cfg):
    """Shard edges by destination, degree-sort nodes into (block, partition)
    slots, lay out payload tiles, fold biases. Returns
    (in_maps, sched, TOT, perms)."""
    C, NSH, NB, M, CH, D = cfg.C, cfg.NSH, cfg.NB, cfg.M, cfg.CH, cfg.D

    sides = [
        (np.asarray(inputs["e_s2d_dst"]), np.asarray(inputs["v_s2d"], np.float32)),
        (np.asarray(inputs["e_d2s_dst"]), np.asarray(inputs["v_d2s"], np.float32)),
    ]

    percore = [[None] * 2 for _ in range(C)]
    for s, (idx_all, v_all) in enumerate(sides):
        core_of = idx_all // NSH
        for c in range(C):
            esel = np.flatnonzero(core_of == c)
            loc = idx_all[esel] - c * NSH
            cnt = np.bincount(loc, minlength=NSH)
            order = np.argsort(-cnt, kind="stable")
            percore[c][s] = (esel, loc, cnt, order)

    # SPMD envelope: per-block tile count = max over cores of block max count
    T = np.zeros((2, NB), np.int64)
    for s in range(2):
        for c in range(C):
            cnt, order = percore[c][s][2], percore[c][s][3]
            sc = cnt[order]
            for b in range(NB):
                T[s][b] = max(T[s][b], sc[128 * b])
    T = np.maximum(T, 1)
    sched = tuple(int(x) for x in T.reshape(-1))
    block_off = np.zeros((2, NB), np.int64)
    off = 0
    for s in range(2):
        for b in range(NB):
            block_off[s][b] = off
            off += T[s][b]
    TILES = off
    NCHUNK = -(-TILES // CH)
    TOT = NCHUNK * CH

    semb = np.asarray(inputs["src_embed"], np.float32)
    demb = np.asarray(inputs["dst_embed"], np.float32)
    emb_by_side = [demb, semb]     # side 0 (s2d) -> dst nodes, side 1 -> src

    def f32(k):
        return np.asarray(inputs[k], np.float32)

    # side-stacked folded weights (side 0 = "col"/dst, side 1 = "row"/src)
    Wside = np.stack([f32("W_dst"), f32("W_src")]).astype(BF16_NP)
    Wmh = np.stack([
        np.vstack([f32("W_sm"), f32("b_sm")[None]]),
        np.vstack([f32("W_dm"), f32("b_dm")[None]]),
    ]).astype(BF16_NP)                                   # [2, M+1, D]
    W1 = np.stack([f32("col_W1"), f32("row_W1")]).astype(BF16_NP)
    W2g = np.stack([
        f32("col_g")[:, None] * f32("col_W2"),
        f32("row_g")[:, None] * f32("row_W2"),
    ]).astype(BF16_NP)
    b2p = [f32("col_beta") @ f32("col_W2") + f32("col_b2"),
           f32("row_beta") @ f32("row_W2") + f32("row_b2")]
    bcols = np.stack([f32("b_dst"), f32("b_src"),
                      f32("col_b1"), f32("row_b1")], axis=1)   # [128, 4]
    identb = np.eye(128, dtype=F8_NP)
    identb16 = np.eye(128, dtype=BF16_NP)

    common = {"identb": identb, "identb16": identb16, "Wside_b": Wside,
              "Wmh_b": Wmh, "W1_b": W1, "W2g_b": W2g,
              "bcols": np.ascontiguousarray(bcols)}

    in_maps = []
    perms = []
    for c in range(C):
        vtiles = np.zeros((TOT, 128, M), F8_NP)
        cntR = np.zeros((2, 1, NB, 128), BF16_NP)
        embT16 = np.zeros((2, D, NSH), BF16_NP)
        embTb2 = np.zeros((2, D, NSH), np.float32)
        ords = []
        for s, (idx_all, v_all) in enumerate(sides):
            esel, loc, cnt, order = percore[c][s]
            ords.append(order)
            pos = np.empty(NSH, np.int64)
            pos[order] = np.arange(NSH)
            eorder = np.argsort(loc, kind="stable")
            starts = np.zeros(NSH + 1, np.int64)
            np.cumsum(cnt, out=starts[1:])
            rank = np.arange(len(eorder)) - starts[loc[eorder]]
            p_of = pos[loc[eorder]]
            tile_of = block_off[s][p_of // 128] + rank
            flat = tile_of * 128 + (p_of % 128)
            vtiles.reshape(-1, M)[flat] = v_all[esel[eorder]].astype(F8_NP)
            cntR[s, 0] = cnt[order].astype(BF16_NP).reshape(NB, 128)
            embT = emb_by_side[s][c * NSH:(c + 1) * NSH][order].T
            embT16[s] = embT.astype(BF16_NP)
            embTb2[s] = embT + b2p[s][:, None]
        m = dict(common)
        m["vhw"] = np.ascontiguousarray(
            vtiles.reshape(NCHUNK, CH, 128, M).transpose(0, 2, 1, 3)
            .reshape(NCHUNK, 128, CH * M))
        m["cntR"] = cntR
        m["embT16"] = embT16
        m["embTb2"] = embTb2
        in_maps.append(m)
        perms.append(ords)
    return in_maps, sched, TOT, perms


# ---------------- device kernel ----------------

def build_kernel(cfg, sched, TOT, reps=1, mode="full"):
    import contextlib
    C, D, M, NSH, NB, CH = cfg.C, cfg.D, cfg.M, cfg.NSH, cfg.NB, cfg.CH
    T = np.asarray(sched, np.int64).reshape(2, NB)
    NCHUNK = TOT // CH
    nc = bacc.Bacc("TRN2", target_bir_lowering=False, debug=False, num_devices=C)

    vhw = nc.dram_tensor("vhw", [NCHUNK, 128, CH * M], F8, kind="ExternalInput")
    identb_d = nc.dram_tensor("identb", [128, 128], F8, kind="ExternalInput")
    identb16_d = nc.dram_tensor("identb16", [128, 128], BF16, kind="ExternalInput")
    Wside_d = nc.dram_tensor("Wside_b", [2, D, D], BF16, kind="ExternalInput")
    Wmh_d = nc.dram_tensor("Wmh_b", [2, M + 1, D], BF16, kind="ExternalInput")
    W1_d = nc.dram_tensor("W1_b", [2, D, D], BF16, kind="ExternalInput")
    W2g_d = nc.dram_tensor("W2g_b", [2, D, D], BF16, kind="ExternalInput")
    bcols_d = nc.dram_tensor("bcols", [128, 4], F32, kind="ExternalInput")
    cntR_d = nc.dram_tensor("cntR", [2, 1, NB, 128], BF16, kind="ExternalInput")
    embT16_d = nc.dram_tensor("embT16", [2, D, NSH], BF16, kind="ExternalInput")
    embTb2_d = nc.dram_tensor("embTb2", [2, D, NSH], F32, kind="ExternalInput")
    rowo = nc.dram_tensor("rowo", [D, NSH], F32, kind="ExternalOutput")
    colo = nc.dram_tensor("colo", [D, NSH], F32, kind="ExternalOutput")

    with tile.TileContext(nc) as tc:
        with (
            tc.tile_pool(name="const", bufs=1) as const,
            tc.tile_pool(name="io", bufs=3) as io,
            tc.tile_pool(name="fin", bufs=2) as fin,
            tc.tile_pool(name="psa", bufs=2, space="PSUM") as psa,
            tc.tile_pool(name="psb", bufs=1, space="PSUM") as psb,
            tc.tile_pool(name="psw", bufs=1, space="PSUM") as psw,
        ):
            identb = const.tile([128, 128], F8)
            nc.sync.dma_start(identb[:], identb_d.ap())
            identb16 = const.tile([128, 128], BF16)
            nc.sync.dma_start(identb16[:], identb16_d.ap())
            eps = const.tile([128, 1], F32)
            nc.vector.memset(eps[:], 1e-5)

            def load2(dram, shp, tag):
                ts = []
                for s in range(2):
                    t = const.tile(shp, BF16, tag=f"{tag}{s}")
                    nc.sync.dma_start(t[:], dram.ap()[s])
                    ts.append(t)
                return ts

            Wside_sb = load2(Wside_d, [D, D], "Wside")
            Wmh_sb = load2(Wmh_d, [M + 1, D], "Wmh")
            W1_sb = load2(W1_d, [D, D], "W1")
            W2g_sb = load2(W2g_d, [D, D], "W2g")
            bcols = const.tile([128, 4], F32)
            nc.sync.dma_start(bcols[:], bcols_d.ap())
            embT16_sb = const.tile([128, 2, NSH], BF16)
            nc.sync.dma_start(embT16_sb[:], embT16_d.ap().rearrange(
                "s d n -> d s n"))
            embTb2_sb = const.tile([128, 2, NSH], F32)
            nc.sync.dma_start(embTb2_sb[:], embTb2_d.ap().rearrange(
                "s d n -> d s n"))
            # AT tiles: rows 0..63 written per side per rep; row 64 = cnt
            AT_sb = []
            for s in range(2):
                t = const.tile([M + 1, NB, 128], BF16, tag=f"AT{s}")
                nc.sync.dma_start(t[M:M + 1, :, :], cntR_d.ap()[s])
                AT_sb.append(t)

            rep_ctx = tc.For_i(0, reps) if reps > 1 else contextlib.nullcontext()
            with rep_ctx:
                run_body(nc, tc, cfg, T, io, fin, psa, psb, psw,
                         vhw, identb, identb16, eps, Wside_sb, Wmh_sb,
                         W1_sb, W2g_sb, bcols, embT16_sb, embTb2_sb,
                         AT_sb, colo, rowo, mode)

    nc.compile()
    return nc


def run_body(nc, tc, cfg, T, io, fin, psa, psb, psw, vhw, identb, identb16,
             eps, Wside_sb, Wmh_sb, W1_sb, W2g_sb, bcols, embT16_sb,
             embTb2_sb, AT_sb, colo, rowo, mode="full"):
    D, M, NB, CH = cfg.D, cfg.M, cfg.NB, cfg.CH
    NW = NB * 128          # nodes per side (1024)
    cur_chunk = [None, -1]
    ntiles_all = int(T.sum())

    def chunk_for(tidx):
        cidx = tidx // CH
        if cur_chunk[1] != cidx:
            t = io.tile([128, CH, M], F8, tag="vchunk")
            n = min(CH, ntiles_all - cidx * CH)
            nc.sync.dma_start(
                t[:, :n, :],
                vhw.ap()[cidx][:, :n * M].rearrange("p (t m) -> p t m", m=M))
            cur_chunk[0], cur_chunk[1] = t, cidx
        return cur_chunk[0]

    if mode == "empty":
        z = fin.tile([128, 1], F32, tag="z")
        nc.vector.memset(z[:], 0.0)
        return

    if mode == "dma":
        for tidx in range(ntiles_all):
            chunk_for(tidx)
        return

    tidx = 0
    for s in range(2):
        out_d = colo if s == 0 else rowo

        # ---- segment sums for all 8 blocks into one PSUM bank ----
        A_w = psa.tile([128, NB * M], F32, tag="A")
        for b in range(NB):
            Tb = int(T[s][b])
            for t in range(Tb):
                ck = chunk_for(tidx)
                nc.tensor.matmul(
                    A_w[:, b * M:(b + 1) * M], lhsT=identb[:],
                    rhs=ck[:, tidx % CH, :],
                    start=(t == 0), stop=(t == Tb - 1),
                    skip_group_check=True)
                tidx += 1

        A_sb = fin.tile([128, NB, M], BF16, tag="Asb")
        nc.vector.tensor_copy(
            A_sb[:].rearrange("p b m -> p (b m)"), A_w[:])

        if mode == "main":
            af = fin.tile([128, NB * M], F32, tag="Af")
            nc.vector.tensor_copy(af[:], A_w[:])
            nc.sync.dma_start(out_d.ap()[:, :NB * M // 2],
                              af[:, :NB * M // 2])
            continue

        # ---- A^T via PE transposes -> AT rows 0..63 (row 64 = cnt) ----
        at_w = psb.tile([M, NB * 128], BF16, tag="tr1")
        for b in range(NB):
            nc.tensor.transpose(
                at_w[:, b * 128:(b + 1) * 128], A_sb[:, b, :], identb16[:])
        nc.scalar.activation(
            AT_sb[s][:M, :, :].rearrange("m b n -> m (b n)"), at_w[:],
            func=mybir.ActivationFunctionType.Copy)

        # ---- h^T = (emb @ Wside)^T ; S^T = (A @ Wm + cnt*bm)^T ----
        h_ps = psw.tile([128, 2, 512], F32, tag="mm1")
        s_ps = psw.tile([128, 2, 512], F32, tag="mm2")
        for j in range(2):
            nc.tensor.matmul(
                h_ps[:, j, :], lhsT=Wside_sb[s][:],
                rhs=embT16_sb[:, s, j * 512:(j + 1) * 512])
            nc.tensor.matmul(
                s_ps[:, j, :], lhsT=Wmh_sb[s][:],
                rhs=AT_sb[s][:].rearrange("m b n -> m (b n)")[
                    :, j * 512:(j + 1) * 512])
        h2 = fin.tile([128, NW], F32, tag="h2")
        nc.vector.tensor_scalar_add(
            h2[:], h_ps[:].rearrange("p j n -> p (j n)"),
            scalar1=bcols[:, s:s + 1])
        u_sb = fin.tile([128, NW], BF16, tag="u")
        nc.vector.tensor_mul(
            u_sb[:], h2[:], s_ps[:].rearrange("p j n -> p (j n)"))

        # ---- t1^T = (u @ W1)^T + b1 ----
        t1_ps = psw.tile([128, 2, 512], F32, tag="mm1")
        for j in range(2):
            nc.tensor.matmul(t1_ps[:, j, :], lhsT=W1_sb[s][:],
                             rhs=u_sb[:, j * 512:(j + 1) * 512])
        t1T = fin.tile([128, NW], BF16, tag="t1T")
        nc.vector.tensor_scalar_add(
            t1T[:], t1_ps[:].rearrange("p j n -> p (j n)"),
            scalar1=bcols[:, 2 + s:3 + s])

        # ---- LayerNorm stats in node orientation ----
        t1w = psb.tile([128, NW], BF16, tag="tr2")
        for b in range(NB):
            nc.tensor.transpose(
                t1w[:, b * 128:(b + 1) * 128],
                t1T[:, b * 128:(b + 1) * 128], identb16[:])
        t1n = fin.tile([128, NB, 128], BF16, tag="t1n")
        nc.scalar.activation(
            t1n[:].rearrange("p b d -> p (b d)"), t1w[:],
            func=mybir.ActivationFunctionType.Copy)
        sum_t = fin.tile([128, NB], F32, tag="sum")
        nc.vector.tensor_reduce(sum_t[:], t1n[:], axis=mybir.AxisListType.X,
                                op=mybir.AluOpType.add)
        sq = fin.tile([128, NB, 128], BF16, tag="sq")
        nc.scalar.activation(sq[:], t1n[:],
                             func=mybir.ActivationFunctionType.Square)
        ssq = fin.tile([128, NB], F32, tag="ssq")
        nc.vector.tensor_reduce(ssq[:], sq[:], axis=mybir.AxisListType.X,
                                op=mybir.AluOpType.add)
        mu = fin.tile([128, NB], F32, tag="mu")
        nc.vector.tensor_scalar_mul(mu[:], in0=sum_t[:], scalar1=1.0 / D)
        var = fin.tile([128, NB], F32, tag="var")
        nc.vector.tensor_scalar_mul(var[:], in0=ssq[:], scalar1=1.0 / D)
        m2 = fin.tile([128, NB], F32, tag="m2")
        nc.vector.tensor_mul(m2[:], mu[:], mu[:])
        nc.vector.tensor_sub(var[:], var[:], m2[:])
        rstd = fin.tile([128, NB], F32, tag="rstd")
        nc.scalar.activation(rstd[:], var[:],
                             func=mybir.ActivationFunctionType.Sqrt,
                             bias=eps[:], scale=1.0)
        nc.vector.reciprocal(rstd[:], rstd[:])
        that = fin.tile([128, NB, 128], BF16, tag="that")
        for b in range(NB):
            nc.vector.tensor_scalar(
                that[:, b, :], in0=t1n[:, b, :],
                scalar1=mu[:, b:b + 1], scalar2=rstd[:, b:b + 1],
                op0=mybir.AluOpType.subtract, op1=mybir.AluOpType.mult)

        # ---- t2^T = (that @ W2g)^T ; out^T = t2^T + embT + b2p ----
        tt_w = psb.tile([128, NW], BF16, tag="tr2")
        for b in range(NB):
            nc.tensor.transpose(
                tt_w[:, b * 128:(b + 1) * 128], that[:, b, :], identb16[:])
        tT = fin.tile([128, NW], BF16, tag="tT")
        nc.scalar.activation(tT[:], tt_w[:],
                             func=mybir.ActivationFunctionType.Copy)
        t2_ps = psw.tile([128, 2, 512], F32, tag="mm2")
        for j in range(2):
            nc.tensor.matmul(t2_ps[:, j, :], lhsT=W2g_sb[s][:],
                             rhs=tT[:, j * 512:(j + 1) * 512])
        ot = fin.tile([128, NW], F32, tag="ot")
        nc.vector.tensor_add(
            ot[:], t2_ps[:].rearrange("p j n -> p (j n)"),
            embTb2_sb[:, s, :])
        nc.sync.dma_start(out_d.ap(), ot[:])


def assemble(results, perms, cfg):
    NSH = cfg.NSH
    row = np.empty((cfg.N, cfg.D), np.float32)
    col = np.empty((cfg.N, cfg.D), np.float32)
    for c, r in enumerate(results):
        ord_s2d, ord_d2s = perms[c]
        col[c * NSH + ord_s2d] = r["colo"].T
        row[c * NSH + ord_d2s] = r["rowo"].T
    return row, col


# ---------------- graded entry point ----------------

_CACHE = {}


def kernel(**inputs):
    cfg = Cfg()
    in_maps, sched, TOT, perms = host_prep(inputs, cfg)
    key = (sched, TOT)
    if key not in _CACHE:
        _CACHE[key] = build_kernel(cfg, sched, TOT)
    nc = _CACHE[key]
    from concourse.bass_utils import run_bass_kernel_spmd
    res = run_bass_kernel_spmd(nc, in_maps, core_ids=list(range(cfg.C)))
    return assemble(res.results, perms, cfg)
